# revision 17
# baseline (speedup 1.0000x reference)
"""Multi-head attention (qk-norm variant) on 8 TRN2 NeuronCores.

Sharding (Megatron-style, per spec hint): core c handles batch b=c//4 and
head-group hg=c%4 (4 of 16 heads). QKV is column-parallel, attention is fully
local per (b, head), and the output projection is row-parallel: each core
produces a partial [N, DIM] output which the host sums per batch and adds
b_proj.

Per-core kernel (bf16 compute, fp32 PSUM accumulation). v5 design:
  - The ACT engine's exp stream (128 calls x ~1.15us over the [128,2,512]
    score tiles) is the roofline; everything else is scheduled to feed it
    without gaps:
      * per key-tile emission slots: scores pair -> exp -> PV of the previous
        unit (shifted 6 slots so the previous normalize can free its psum)
        -> one slice of "filler" PE work (v projection, qkv for heads 2/3,
        output projection), sized to keep the per-slot PE time under the
        1.15us exp cadence so the tensor engine stays busy (HAM stays warm)
        but never starves ACT.
  - qk-norm rstd has NO scalar-engine work and NO serial DMA reshape chain:
    sum-of-squares comes from a TRANSPOSED ones matmul (lhsT = squares tile,
    rhs = O2) that lands token-on-partitions [128,(kt,head)] directly; rstd =
    1/sqrt(ms/64+eps) via cubic seed + 3 Newton steps on DVE; the only DMA is
    one scatter to DRAM + one partition-replicating load back (64-way
    broadcast), issued on the ACT queue for the prologue groups (idle before
    the exp stream) and on the gpsimd queue for heads 2/3 (so the Sync and
    ACT queues never head-of-line block on the LN chain).
  - weights are pre-centered on the host (LN mean subtraction is linear);
    softmax needs no max-subtraction (|logits| <= 8 after qk-norm).
  - v is token-major with a fused ones column: the PV matmul (M=65) yields
    the softmax denominator as psum row 64 for free.
"""
import numpy as np
import ml_dtypes

import concourse.bass as bass
import concourse.bacc as bacc
import concourse.tile as tile
from concourse import mybir
from concourse.bass_utils import run_bass_kernel_spmd

F32 = mybir.dt.float32
BF16 = mybir.dt.bfloat16
AF = mybir.ActivationFunctionType
ALU = mybir.AluOpType

B, N, DIM = 2, 2048, 1024
H, D = 16, 64
EPS = 1e-5
N_CORES = 8
HPC = 4              # heads per core
HF = HPC * D         # 256 local head features
KT = DIM // 128      # 8 contraction tiles
NT = N // 128        # 16 token tiles
NCH = N // 512       # 4 token chunks
SCALE = D ** -0.5

# rsqrt seed polynomial (relative-error weighted cubic fit on [0.2, 3.5];
# with 3 Newton steps max rel err < 5e-5 on the realistic ms range)
_tt = np.linspace(0.2, 3.5, 4000)
_RC = np.polyfit(_tt, _tt ** -0.5, 3, w=_tt ** 0.5)

# set by test harness to request NTFF profiling
TRACE = False
LAST_EXEC_NS = None
LAST_RESULTS = None

_BUILD_CACHE = {}


def _build(has_qkbias, has_qgamma, has_kgamma, has_qbeta, has_kbeta,
           has_vbias):
    key = (has_qkbias, has_qgamma, has_kgamma, has_qbeta, has_kbeta,
           has_vbias)
    if key in _BUILD_CACHE:
        return _BUILD_CACHE[key]

    nc = bacc.Bacc("TRN2", target_bir_lowering=False, debug=False,
                   num_devices=N_CORES)

    # all inputs arrive partition-major so each DMA is one fat descriptor
    # with long contiguous runs per partition
    xT_d = nc.dram_tensor("xT", [128, KT, N], BF16, kind="ExternalInput")
    # columns ordered [q01 | k01 | q23 | k23] so group g = cols g*128..
    wqkT_d = nc.dram_tensor("wqkT", [128, KT, 2 * HF], BF16,
                            kind="ExternalInput")
    wvT_d = nc.dram_tensor("wvT", [128, KT, HF], BF16, kind="ExternalInput")
    wpT_d = nc.dram_tensor("wpT", [128, 2, DIM], BF16, kind="ExternalInput")
    eye_d = nc.dram_tensor("eye128", [128, 128], BF16, kind="ExternalInput")
    bqk_d = ones_d = bvT_d = gamma_d = beta_d = None
    if has_qkbias:
        bqk_d = nc.dram_tensor("bqk_cols", [128, 4], F32, kind="ExternalInput")
    if has_vbias:
        bvT_d = nc.dram_tensor("bvT", [1, HF], BF16, kind="ExternalInput")
        ones_d = nc.dram_tensor("ones512", [1, 512], BF16, kind="ExternalInput")
    if has_qgamma or has_kgamma:
        gamma_d = nc.dram_tensor("gamma_cols", [128, 2], F32, kind="ExternalInput")
    if has_qbeta or has_kbeta:
        beta_d = nc.dram_tensor("beta_cols", [128, 2], F32, kind="ExternalInput")
    out_d = nc.dram_tensor("out_partial", [N, DIM], BF16, kind="ExternalOutput")

    with tile.TileContext(nc) as tc:
        with (
            tc.tile_pool(name="persist", bufs=1) as pp,
            tc.tile_pool(name="work", bufs=2) as wp,
            tc.tile_pool(name="psum", bufs=1, space="PSUM") as psp,
            tc.tile_pool(name="dram", bufs=1, space="DRAM") as dp,
        ):
            # ---- persistent SBUF tensors ----
            xT = pp.tile([128, KT, N], BF16)
            wqk = pp.tile([128, KT, 2 * HF], BF16)
            wv = pp.tile([128, KT, HF], BF16)
            wpj = pp.tile([128, 2, DIM], BF16)
            O2 = pp.tile([128, 2], BF16)
            eye128 = pp.tile([128, 128], BF16)
            bqk = pp.tile([128, 4], F32) if bqk_d is not None else None
            bvT = pp.tile([1, HF], BF16) if bvT_d is not None else None
            ones512 = pp.tile([1, 512], BF16) if ones_d is not None else None
            gamma_c = pp.tile([128, 2], F32) if gamma_d is not None else None
            beta_c = pp.tile([128, 2], F32) if beta_d is not None else None

            # v token-major with a ones column at index 64
            v_sb = pp.tile([128, NT, HPC, 66], BF16)
            # q/k head-major, groups g: 0=q01 1=k01 2=q23 3=k23
            qkt = pp.tile([128, 4, N], BF16)
            outT_n = pp.tile([128, 2, N], BF16)   # attn out, head-major

            # rstd DRAM bounce target, [head j, kt, token-in-tile] per group
            rstd_g = [dp.tile([2, 16, 128], BF16, name=f"rstd{g}")
                      for g in range(4)]

            # ---- input DMA: few big descriptors, parallel rings ----
            xv = xT_d.ap()
            nc.sync.dma_start(out=xT[:, 0:4, :], in_=xv[:, 0:4, :])
            nc.sync.dma_start(out=xT[:, 4:8, :], in_=xv[:, 4:8, :])
            nc.sync.dma_start(out=wqk, in_=wqkT_d.ap())
            nc.sync.dma_start(out=wv, in_=wvT_d.ap())
            nc.sync.dma_start(out=wpj, in_=wpT_d.ap())
            nc.sync.dma_start(out=eye128, in_=eye_d.ap())
            for t, d in [(bqk, bqk_d), (bvT, bvT_d), (ones512, ones_d),
                         (gamma_c, gamma_d), (beta_c, beta_d)]:
                if t is not None:
                    nc.sync.dma_start(out=t, in_=d.ap())

            nc.vector.memset(v_sb[:, :, :, 64:66], 0.0)
            nc.vector.memset(v_sb[:, :, :, 64:65], 1.0)
            nc.vector.memset(O2[0:64, 0:1], 1.0)
            nc.vector.memset(O2[64:128, 0:1], 0.0)
            nc.vector.memset(O2[0:64, 1:2], 0.0)
            nc.vector.memset(O2[64:128, 1:2], 1.0)

            sq_g = {}
            rb_g = {}

            def _g_tiles(g):
                if g not in sq_g:
                    sq_g[g] = wp.tile([128, NCH, 512], BF16, tag="sq", bufs=2,
                                      name=f"sq{g}")
                    rb_g[g] = wp.tile([128, N], BF16, tag="rb", bufs=2,
                                      name=f"rb{g}")

            def qk_post(g, ch, ps):
                """PSUM -> qkt copy and square for one finished chunk."""
                csl = slice(ch * 512, (ch + 1) * 512)
                if has_qkbias:
                    nc.vector.tensor_scalar_add(
                        qkt[:, g, csl], ps, bqk[:, g:g + 1])
                else:
                    nc.vector.tensor_copy(qkt[:, g, csl], ps)
                nc.vector.tensor_mul(sq_g[g][:, ch, :], qkt[:, g, csl],
                                     qkt[:, g, csl])

            def qk_chunk_pair(g, cp):
                """two chunks (2cp, 2cp+1) of group g in one score-tag psum
                tile (prologue; shared lhsT per kt amortizes weight loads)."""
                _g_tiles(g)
                ps2 = psp.tile([128, 2, 512], F32, tag="score", bufs=2,
                               name="ps_qk2")
                for kt in range(KT):
                    for i in range(2):
                        csl = slice((2 * cp + i) * 512, (2 * cp + i + 1) * 512)
                        nc.tensor.matmul(
                            ps2[:, i, :],
                            wqk[:, kt, g * 128:(g + 1) * 128],
                            xT[:, kt, csl],
                            start=(kt == 0), stop=(kt == KT - 1))
                for i in range(2):
                    qk_post(g, 2 * cp + i, ps2[:, i, :])

            def qk_chunk(g, ch):
                """single-chunk qkv on the misc tag (mid-stream filler)."""
                _g_tiles(g)
                csl = slice(ch * 512, (ch + 1) * 512)
                ps_qk = psp.tile([128, 512], F32, tag="misc", bufs=2,
                                 name="ps_qk")
                for kt in range(KT):
                    nc.tensor.matmul(
                        ps_qk,
                        wqk[:, kt, g * 128:(g + 1) * 128],
                        xT[:, kt, csl],
                        start=(kt == 0), stop=(kt == KT - 1))
                qk_post(g, ch, ps_qk)

            def ln_group(g, eng):
                """rstd for group g: transposed ssq matmuls land ms token-on-
                partitions (no reshape DMA); Newton rsqrt on DVE; one scatter
                + two replicating loads on `eng`'s DMA queue."""
                ps_ms = psp.tile([128, 512], F32, tag="misc", bufs=2,
                                 name="ps_mst")
                msv = ps_ms.rearrange("p (c j) -> p c j", j=2)
                for tt in range(NT):
                    nc.tensor.matmul(
                        msv[:, tt, :],
                        sq_g[g][:, tt // 4, (tt % 4) * 128:(tt % 4 + 1) * 128],
                        O2, start=True, stop=True)
                nt_in = wp.tile([128, 32], F32, tag="nt_in", bufs=2)
                nc.vector.tensor_copy(nt_in, ps_ms[:, 0:32])
                t = wp.tile([128, 32], F32, tag="nt_t", bufs=2)
                nc.vector.tensor_scalar(t, nt_in, 1.0 / D, EPS, ALU.mult,
                                        ALU.add)
                y = wp.tile([128, 32], F32, tag="nt_y", bufs=2)
                a = wp.tile([128, 32], F32, tag="nt_a", bufs=2)
                nc.vector.tensor_scalar(y, t, float(_RC[0]), float(_RC[1]),
                                        ALU.mult, ALU.add)
                nc.vector.tensor_mul(y, y, t)
                nc.vector.tensor_scalar_add(y, y, float(_RC[2]))
                nc.vector.tensor_mul(y, y, t)
                nc.vector.tensor_scalar_add(y, y, float(_RC[3]))
                nc.vector.tensor_scalar(y, y, 0.1, 2.4, ALU.max, ALU.min)
                nt_out = wp.tile([128, 2, 16], BF16, tag="nt_out", bufs=2)
                for it in range(2):
                    nc.vector.tensor_mul(a, y, y)
                    nc.vector.tensor_mul(a, a, t)
                    nc.vector.tensor_scalar(a, a, -0.5, 1.5, ALU.mult, ALU.add)
                    if it < 1:
                        nc.vector.tensor_mul(y, y, a)
                    else:
                        nc.vector.tensor_mul(
                            nt_out.rearrange("p j c -> p c j"),
                            y.rearrange("p (c j) -> p c j", j=2),
                            a.rearrange("p (c j) -> p c j", j=2))
                # transpose on PE so both DMA hops stay contiguous:
                # rows_sb[(j,c), tok] -> DRAM rows -> 64-way replicated load
                ps_tr = psp.tile([128, 512], F32, tag="misc", bufs=2,
                                 name="ps_tr")
                nc.tensor.transpose(ps_tr.bitcast(BF16)[0:32, 0:128],
                                    nt_out.rearrange("p j c -> p (j c)"),
                                    eye128)
                rows_sb = wp.tile([32, 128], BF16, tag="rows", bufs=2)
                nc.vector.tensor_copy(rows_sb,
                                      ps_tr.bitcast(BF16)[0:32, 0:128])
                eng.dma_start(out=rstd_g[g].rearrange("j c p -> (j c) p"),
                              in_=rows_sb)
                for j in range(2):
                    row = rstd_g[g][j:j + 1, :, :].rearrange("j c p -> j (c p)")
                    bc = bass.AP(tensor=row.tensor, offset=row.offset,
                                 ap=[[0, 64]] + list(row.ap[1:]))
                    eng.dma_start(out=rb_g[g][64 * j:64 * (j + 1), :], in_=bc)

            def apply_chunk(g, ch):
                """multiply qkt chunk by its per-token rstd broadcast."""
                csl = slice(ch * 512, (ch + 1) * 512)
                nc.vector.tensor_mul(qkt[0:64, g, csl], qkt[0:64, g, csl],
                                     rb_g[g][0:64, csl])
                nc.vector.tensor_mul(qkt[64:128, g, csl], qkt[64:128, g, csl],
                                     rb_g[g][64:128, csl])
                is_q = (g % 2 == 0)
                gcol = None
                if is_q and has_qgamma:
                    gcol = gamma_c[:, 0:1]
                elif not is_q and has_kgamma:
                    gcol = gamma_c[:, 1:2]
                bcol = None
                if is_q and has_qbeta:
                    bcol = beta_c[:, 0:1]
                elif not is_q and has_kbeta:
                    bcol = beta_c[:, 1:2]
                if gcol is not None:
                    nc.vector.tensor_scalar_mul(qkt[:, g, csl],
                                                qkt[:, g, csl], gcol)
                if bcol is not None:
                    nc.vector.tensor_scalar_add(qkt[:, g, csl],
                                                qkt[:, g, csl], bcol)

            def v_feats(tt):
                """v token-major projection for token tile tt."""
                tsl = slice(tt * 128, (tt + 1) * 128)
                ps_v = psp.tile([128, 512], F32, tag="misc", bufs=2,
                                name="ps_v")
                for kt in range(KT):
                    nc.tensor.matmul(
                        ps_v[:, 0:HF], xT[:, kt, tsl], wv[:, kt, :],
                        start=(kt == 0),
                        stop=(not has_vbias and kt == KT - 1))
                if has_vbias:
                    nc.tensor.matmul(ps_v[:, 0:HF], ones512[:, 0:128],
                                     bvT, start=False, stop=True)
                nc.vector.tensor_copy(
                    v_sb[:, tt, :, 0:64],
                    ps_v[:, 0:HF].rearrange("p (h d) -> p h d", h=HPC))

            def normalize(pgq, pqc, pouts):
                """divide PV psum by the fused denominator row, write outT.
                (reciprocal_approx_fast misreads PSUM sources - stage the
                denominator row through SBUF first)"""
                qsl = slice(pqc * 512, (pqc + 1) * 512)
                for hp in range(2):
                    p0 = hp * 64
                    ps_o = pouts[hp]
                    den = wp.tile([1, 512], F32, tag="den", bufs=3)
                    nc.vector.tensor_copy(den, ps_o[64:65, :])
                    rec = wp.tile([1, 512], F32, tag="rec", bufs=3)
                    nc.vector.reciprocal_approx_fast(rec, den)
                    rb2 = wp.tile([64, 512], F32, tag="rb2", bufs=3)
                    nc.gpsimd.partition_broadcast(rb2, rec)
                    nc.vector.tensor_mul(outT_n[p0:p0 + 64, pgq, qsl],
                                         ps_o[0:64, :], rb2)

            def proj_tile(tt):
                """output projection for one token tile (two 512-wide halves
                of DIM in one score-tag psum tile)."""
                tsl = slice(tt * 128, (tt + 1) * 128)
                ps_p = psp.tile([128, 2, 512], F32, tag="score", bufs=2,
                                name="ps_p")
                for fn in range(2):
                    fsl = slice(fn * 512, (fn + 1) * 512)
                    for t in range(2):
                        nc.tensor.matmul(ps_p[:, fn, :],
                                         outT_n[:, t, tsl],
                                         wpj[:, t, fsl],
                                         start=(t == 0), stop=(t == 1))
                ostg = wp.tile([128, DIM], BF16, tag="ostg", bufs=3)
                nc.vector.tensor_copy(ostg, ps_p.rearrange("p a b -> p (a b)"))
                nc.sync.dma_start(out=out_d.ap()[tsl, :], in_=ostg)

            def unit(gq, qc, prev, fillers=None, shift=6, trail=False):
                """scores+exp for unit (gq, qc). PV of `prev` rides `shift`
                slots behind; `fillers[kt]` emits extra PE work at slot kt;
                with trail=True this unit's own PV rides 1 slot behind on
                misc-tag psum (last unit only)."""
                qg, kg = (0, 1) if gq == 0 else (2, 3)
                qsl = slice(qc * 512, (qc + 1) * 512)
                exp_pair = wp.tile([128, NT, 2, 512], BF16, tag="exp",
                                   bufs=2, name="exp_pair")
                pouts = touts = None
                if prev is not None:
                    pgq, pqc, pexp = prev
                    pouts = [psp.tile([65, 512], F32, tag="pvc", bufs=2,
                                      name=f"ps_o{hp}") for hp in range(2)]
                if trail:
                    touts = [psp.tile([65, 512], F32, tag="pvc", bufs=2,
                                      name=f"ps_t{hp}") for hp in range(2)]

                def pv_kt(kt):
                    for hp in range(2):
                        nc.tensor.matmul(
                            pouts[hp], v_sb[:, kt, 2 * pgq + hp, 0:65],
                            pexp[:, kt, hp, :],
                            start=(kt == 0), stop=(kt == NT - 1))

                def pvt_kt(kt):
                    for hp in range(2):
                        nc.tensor.matmul(
                            touts[hp], v_sb[:, kt, 2 * gq + hp, 0:65],
                            exp_pair[:, kt, hp, :],
                            start=(kt == 0), stop=(kt == NT - 1))

                for kt in range(NT):
                    ktsl = slice(kt * 128, (kt + 1) * 128)
                    ps_s = psp.tile([128, 2, 512], F32, tag="score",
                                    bufs=2, name="ps_s")
                    for hp in range(2):
                        p0 = hp * 64
                        nc.tensor.matmul(ps_s[:, hp, :],
                                         qkt[p0:p0 + 64, kg, ktsl],
                                         qkt[p0:p0 + 64, qg, qsl],
                                         start=True, stop=True)
                    nc.scalar.activation(exp_pair[:, kt, :, :], ps_s,
                                         AF.Exp, scale=SCALE)
                    if prev is not None:
                        if trail:
                            # dense handoff: prev PV two pairs per slot at
                            # slots 2..9, freeing pvc mid-unit for our own
                            if 2 <= kt <= 9:
                                pv_kt(2 * (kt - 2))
                                pv_kt(2 * (kt - 2) + 1)
                            if kt == 10:
                                normalize(pgq, pqc, pouts)
                        elif kt >= shift:
                            pv_kt(kt - shift)
                    if trail and 11 <= kt:
                        pvt_kt(2 * (kt - 11))
                        pvt_kt(2 * (kt - 11) + 1)
                    if fillers is not None and kt in fillers:
                        fillers[kt]()
                if prev is not None and not trail:
                    for kt in range(NT - shift, NT):
                        pv_kt(kt)
                    normalize(pgq, pqc, pouts)
                if trail:
                    for kt in range(10, NT):
                        pvt_kt(kt)
                    normalize(gq, qc, touts)
                return (gq, qc, exp_pair)

            # ---- emission (priority order = emission order) ----
            with nc.named_scope("prologue"):
                for g in (1, 0):                      # k01 then q01
                    for cp in range(NCH // 2):
                        qk_chunk_pair(g, cp)
                for g in (1, 0):
                    ln_group(g, nc.scalar)            # DMA on idle ACT queue
                for ch in range(NCH):
                    apply_chunk(1, ch)
                    apply_chunk(0, ch)

            with nc.named_scope("attn"):
                u = unit(0, 0, None,
                         fillers={kt: (lambda kt=kt: v_feats(kt))
                                  for kt in range(NT)})
                f01 = {1: lambda: qk_chunk(2, 0), 4: lambda: qk_chunk(2, 1),
                       7: lambda: qk_chunk(2, 2), 10: lambda: qk_chunk(2, 3),
                       13: lambda: ln_group(2, nc.gpsimd)}
                u = unit(0, 1, u, fillers=f01)
                f02 = {2: lambda: qk_chunk(3, 0), 6: lambda: qk_chunk(3, 1),
                       9: lambda: qk_chunk(3, 2), 12: lambda: qk_chunk(3, 3),
                       14: lambda: [apply_chunk(2, c) for c in range(NCH)]}
                u = unit(0, 2, u, fillers=f02)
                f03 = {1: lambda: ln_group(3, nc.gpsimd),
                       9: lambda: [apply_chunk(3, c) for c in range(NCH)]}
                u = unit(0, 3, u, fillers=f03)
                u = unit(1, 0, u)
                u = unit(1, 1, u)
                fp0 = {2: lambda: proj_tile(0), 6: lambda: proj_tile(1),
                       10: lambda: proj_tile(2), 14: lambda: proj_tile(3)}
                u = unit(1, 2, u, fillers=fp0)
                fp1 = {0: lambda: proj_tile(4), 1: lambda: proj_tile(5),
                       10: lambda: proj_tile(6), 14: lambda: proj_tile(7)}
                u = unit(1, 3, u, fillers=fp1, trail=True)
            with nc.named_scope("tail"):
                for tt in range(8, 16):
                    proj_tile(tt)

    nc.compile()
    _BUILD_CACHE[key] = nc
    return nc


def _bf16(a):
    return np.ascontiguousarray(a).astype(ml_dtypes.bfloat16)


def kernel(**inputs):
    global LAST_EXEC_NS
    x = np.asarray(inputs["x"], np.float32)
    w_qkv = np.asarray(inputs["w_qkv"], np.float32)
    b_qkv = np.asarray(inputs["b_qkv"], np.float32)
    q_gamma = np.asarray(inputs["q_gamma"], np.float32)
    q_beta = np.asarray(inputs["q_beta"], np.float32)
    k_gamma = np.asarray(inputs["k_gamma"], np.float32)
    k_beta = np.asarray(inputs["k_beta"], np.float32)
    w_proj = np.asarray(inputs["w_proj"], np.float32)
    b_proj = np.asarray(inputs["b_proj"], np.float32)

    has_qkbias = bool(np.any(b_qkv[0:2 * DIM] != 0.0))
    has_qgamma = not bool(np.all(q_gamma == 1.0))
    has_kgamma = not bool(np.all(k_gamma == 1.0))
    has_qbeta = bool(np.any(q_beta != 0.0))
    has_kbeta = bool(np.any(k_beta != 0.0))
    has_vbias = bool(np.any(b_qkv[2 * DIM:3 * DIM] != 0.0))
    nc = _build(has_qkbias, has_qgamma, has_kgamma, has_qbeta, has_kbeta,
                has_vbias)

    # shared constants
    Cd = np.eye(D, dtype=np.float32) - 1.0 / D   # centering matrix (folded
    eye128 = np.eye(128, dtype=np.float32)       # into the qkv weights below)
    gamma_cols = np.stack([np.tile(q_gamma, 2), np.tile(k_gamma, 2)],
                          axis=1).astype(np.float32)
    ones512 = np.ones((1, 512), np.float32)
    beta_cols = np.stack([np.tile(q_beta, 2), np.tile(k_beta, 2)],
                         axis=1).astype(np.float32)
    # pre-center the q/k projection weights and biases per head:
    # LN(Wx+b) centering is linear, so fold (I - J/64) into W and b
    w_qkv = w_qkv.copy()
    b_qkv = b_qkv.copy()
    for h in range(2 * H):            # 16 q heads then 16 k heads
        rs = slice(h * D, (h + 1) * D)
        w_qkv[rs] = Cd @ w_qkv[rs]
        b_qkv[rs] = Cd @ b_qkv[rs]

    in_maps = []
    for c in range(N_CORES):
        b, hg = divmod(c, 4)
        rows = slice(hg * HF, (hg + 1) * HF)
        q_l = w_qkv[0 * DIM:1 * DIM][rows]           # [256, 1024]
        k_l = w_qkv[1 * DIM:2 * DIM][rows]
        v_l = w_qkv[2 * DIM:3 * DIM][rows]
        bq_l = b_qkv[0 * DIM:1 * DIM][rows]
        bk_l = b_qkv[1 * DIM:2 * DIM][rows]
        bv_l = b_qkv[2 * DIM:3 * DIM][rows]
        # group order [q01 | k01 | q23 | k23]
        wqk_rows = np.concatenate([q_l[:128], k_l[:128],
                                   q_l[128:], k_l[128:]], 0)
        bqk_cols = np.stack([bq_l[:128], bk_l[:128], bq_l[128:], bk_l[128:]],
                            axis=1).astype(np.float32)
        def _pm(a, k):
            # [k*128, c] -> partition-major [128, k, c]
            return _bf16(np.ascontiguousarray(
                a.reshape(k, 128, a.shape[1]).transpose(1, 0, 2)))
        m = {
            "xT": _pm(x[b].T, KT),                        # [128, 8, 2048]
            "wqkT": _pm(wqk_rows.T, KT),                  # [128, 8, 512]
            "wvT": _pm(v_l.T, KT),                        # [128, 8, 256]
            "wpT": _pm(w_proj[:, rows].T, 2),             # [128, 2, 1024]
            "eye128": _bf16(eye128),
        }
        if has_qkbias:
            m["bqk_cols"] = bqk_cols
        if has_vbias:
            m["bvT"] = _bf16(bv_l[None, :])
            m["ones512"] = _bf16(ones512)
        if has_qgamma or has_kgamma:
            m["gamma_cols"] = gamma_cols
        if has_qbeta or has_kbeta:
            m["beta_cols"] = beta_cols
        in_maps.append(m)

    res = run_bass_kernel_spmd(nc, in_maps, core_ids=list(range(N_CORES)),
                               trace=TRACE)
    LAST_EXEC_NS = res.exec_time_ns
    globals()["LAST_RESULTS"] = res

    out = np.zeros((B, N, DIM), np.float32)
    for c in range(N_CORES):
        out[c // 4] += np.asarray(res.results[c]["out_partial"], np.float32)
    out += b_proj[None, None, :]
    return out


# revision 18
# speedup vs baseline: 1.0261x; 1.0261x over previous
"""Multi-head attention (qk-norm variant) on 8 TRN2 NeuronCores.

Sharding (Megatron-style, per spec hint): core c handles batch b=c//4 and
head-group hg=c%4 (4 of 16 heads). QKV is column-parallel, attention is fully
local per (b, head), and the output projection is row-parallel: each core
produces a partial [N, DIM] output which the host sums per batch and adds
b_proj.

Per-core kernel (bf16 compute, fp32 PSUM accumulation). v5 design:
  - The ACT engine's exp stream (128 calls x ~1.15us over the [128,2,512]
    score tiles) is the roofline; everything else is scheduled to feed it
    without gaps:
      * per key-tile emission slots: scores pair -> exp -> PV of the previous
        unit (shifted 6 slots so the previous normalize can free its psum)
        -> one slice of "filler" PE work (v projection, qkv for heads 2/3,
        output projection), sized to keep the per-slot PE time under the
        1.15us exp cadence so the tensor engine stays busy (HAM stays warm)
        but never starves ACT.
  - qk-norm rstd has NO scalar-engine work and NO serial DMA reshape chain:
    sum-of-squares comes from a TRANSPOSED ones matmul (lhsT = squares tile,
    rhs = O2) that lands token-on-partitions [128,(kt,head)] directly; rstd =
    1/sqrt(ms/64+eps) via cubic seed + 3 Newton steps on DVE; the only DMA is
    one scatter to DRAM + one partition-replicating load back (64-way
    broadcast), issued on the ACT queue for the prologue groups (idle before
    the exp stream) and on the gpsimd queue for heads 2/3 (so the Sync and
    ACT queues never head-of-line block on the LN chain).
  - weights are pre-centered on the host (LN mean subtraction is linear);
    softmax needs no max-subtraction (|logits| <= 8 after qk-norm).
  - v is token-major with a fused ones column: the PV matmul (M=65) yields
    the softmax denominator as psum row 64 for free.
"""
import numpy as np
import ml_dtypes

import concourse.bass as bass
import concourse.bacc as bacc
import concourse.tile as tile
from concourse import mybir
from concourse.bass_utils import run_bass_kernel_spmd

F32 = mybir.dt.float32
BF16 = mybir.dt.bfloat16
AF = mybir.ActivationFunctionType
ALU = mybir.AluOpType

B, N, DIM = 2, 2048, 1024
H, D = 16, 64
EPS = 1e-5
N_CORES = 8
HPC = 4              # heads per core
HF = HPC * D         # 256 local head features
KT = DIM // 128      # 8 contraction tiles
NT = N // 128        # 16 token tiles
NCH = N // 512       # 4 token chunks
SCALE = D ** -0.5

# rsqrt seed polynomial (relative-error weighted cubic fit on [0.2, 3.5];
# with 3 Newton steps max rel err < 5e-5 on the realistic ms range)
_tt = np.linspace(0.2, 3.5, 4000)
_RC = np.polyfit(_tt, _tt ** -0.5, 3, w=_tt ** 0.5)

# set by test harness to request NTFF profiling
TRACE = False
LAST_EXEC_NS = None
LAST_RESULTS = None

_BUILD_CACHE = {}


def _build(has_qkbias, has_qgamma, has_kgamma, has_qbeta, has_kbeta,
           has_vbias):
    key = (has_qkbias, has_qgamma, has_kgamma, has_qbeta, has_kbeta,
           has_vbias)
    if key in _BUILD_CACHE:
        return _BUILD_CACHE[key]

    nc = bacc.Bacc("TRN2", target_bir_lowering=False, debug=False,
                   num_devices=N_CORES)

    # all inputs arrive partition-major so each DMA is one fat descriptor
    # with long contiguous runs per partition
    xT_d = nc.dram_tensor("xT", [128, KT, N], BF16, kind="ExternalInput")
    # columns ordered [q01 | k01 | q23 | k23] so group g = cols g*128..
    wqkT_d = nc.dram_tensor("wqkT", [128, KT, 2 * HF], BF16,
                            kind="ExternalInput")
    wvT_d = nc.dram_tensor("wvT", [128, KT, HF], BF16, kind="ExternalInput")
    wpT_d = nc.dram_tensor("wpT", [128, 2, DIM], BF16, kind="ExternalInput")
    eye_d = nc.dram_tensor("eye128", [128, 128], BF16, kind="ExternalInput")
    bqk_d = ones_d = bvT_d = gamma_d = beta_d = None
    if has_qkbias:
        bqk_d = nc.dram_tensor("bqk_cols", [128, 4], F32, kind="ExternalInput")
    if has_vbias:
        bvT_d = nc.dram_tensor("bvT", [1, HF], BF16, kind="ExternalInput")
        ones_d = nc.dram_tensor("ones512", [1, 512], BF16, kind="ExternalInput")
    if has_qgamma or has_kgamma:
        gamma_d = nc.dram_tensor("gamma_cols", [128, 2], F32, kind="ExternalInput")
    if has_qbeta or has_kbeta:
        beta_d = nc.dram_tensor("beta_cols", [128, 2], F32, kind="ExternalInput")
    out_d = nc.dram_tensor("out_partial", [N, DIM], BF16, kind="ExternalOutput")

    with tile.TileContext(nc) as tc:
        with (
            tc.tile_pool(name="persist", bufs=1) as pp,
            tc.tile_pool(name="work", bufs=2) as wp,
            tc.tile_pool(name="psum", bufs=1, space="PSUM") as psp,
            tc.tile_pool(name="dram", bufs=1, space="DRAM") as dp,
        ):
            # ---- persistent SBUF tensors ----
            xT = pp.tile([128, KT, N], BF16)
            wqk = pp.tile([128, KT, 2 * HF], BF16)
            wv = pp.tile([128, KT, HF], BF16)
            wpj = pp.tile([128, 2, DIM], BF16)
            O2 = pp.tile([128, 2], BF16)
            eye128 = pp.tile([128, 128], BF16)
            bqk = pp.tile([128, 4], F32) if bqk_d is not None else None
            bvT = pp.tile([1, HF], BF16) if bvT_d is not None else None
            ones512 = pp.tile([1, 512], BF16) if ones_d is not None else None
            gamma_c = pp.tile([128, 2], F32) if gamma_d is not None else None
            beta_c = pp.tile([128, 2], F32) if beta_d is not None else None

            # v token-major with a ones column at index 64
            v_sb = pp.tile([128, NT, HPC, 66], BF16)
            # q/k head-major, groups g: 0=q01 1=k01 2=q23 3=k23
            qkt = pp.tile([128, 4, N], BF16)
            outT_n = pp.tile([128, 2, N], BF16)   # attn out, head-major

            # rstd DRAM bounce target, [head j, kt, token-in-tile] per group
            rstd_g = [dp.tile([2, 16, 128], BF16, name=f"rstd{g}")
                      for g in range(4)]

            # ---- input DMA: few big descriptors, parallel rings ----
            xv = xT_d.ap()
            nc.sync.dma_start(out=xT[:, 0:4, :], in_=xv[:, 0:4, :])
            nc.sync.dma_start(out=xT[:, 4:8, :], in_=xv[:, 4:8, :])
            nc.sync.dma_start(out=wqk, in_=wqkT_d.ap())
            nc.sync.dma_start(out=wv, in_=wvT_d.ap())
            nc.sync.dma_start(out=wpj, in_=wpT_d.ap())
            nc.sync.dma_start(out=eye128, in_=eye_d.ap())
            for t, d in [(bqk, bqk_d), (bvT, bvT_d), (ones512, ones_d),
                         (gamma_c, gamma_d), (beta_c, beta_d)]:
                if t is not None:
                    nc.sync.dma_start(out=t, in_=d.ap())

            nc.vector.memset(v_sb[:, :, :, 64:66], 0.0)
            nc.vector.memset(v_sb[:, :, :, 64:65], 1.0)
            nc.vector.memset(O2[0:64, 0:1], 1.0)
            nc.vector.memset(O2[64:128, 0:1], 0.0)
            nc.vector.memset(O2[0:64, 1:2], 0.0)
            nc.vector.memset(O2[64:128, 1:2], 1.0)

            sq_g = {}
            rb_g = {}

            def _g_tiles(g):
                if g not in sq_g:
                    sq_g[g] = wp.tile([128, NCH, 512], BF16, tag="sq", bufs=2,
                                      name=f"sq{g}")
                    rb_g[g] = wp.tile([128, N], BF16, tag="rb", bufs=2,
                                      name=f"rb{g}")

            def qk_post(g, ch, ps):
                """PSUM -> qkt copy and square for one finished chunk."""
                csl = slice(ch * 512, (ch + 1) * 512)
                if has_qkbias:
                    nc.vector.tensor_scalar_add(
                        qkt[:, g, csl], ps, bqk[:, g:g + 1])
                else:
                    nc.vector.tensor_copy(qkt[:, g, csl], ps)
                nc.vector.tensor_mul(sq_g[g][:, ch, :], qkt[:, g, csl],
                                     qkt[:, g, csl])

            def qk_chunk_pair(g, cp):
                """two chunks (2cp, 2cp+1) of group g in one score-tag psum
                tile (prologue; shared lhsT per kt amortizes weight loads)."""
                _g_tiles(g)
                ps2 = psp.tile([128, 2, 512], F32, tag="score", bufs=2,
                               name="ps_qk2")
                for kt in range(KT):
                    for i in range(2):
                        csl = slice((2 * cp + i) * 512, (2 * cp + i + 1) * 512)
                        nc.tensor.matmul(
                            ps2[:, i, :],
                            wqk[:, kt, g * 128:(g + 1) * 128],
                            xT[:, kt, csl],
                            start=(kt == 0), stop=(kt == KT - 1))
                for i in range(2):
                    qk_post(g, 2 * cp + i, ps2[:, i, :])

            def qk_chunk(g, ch):
                """single-chunk qkv on the misc tag (mid-stream filler)."""
                _g_tiles(g)
                csl = slice(ch * 512, (ch + 1) * 512)
                ps_qk = psp.tile([128, 512], F32, tag="misc", bufs=2,
                                 name="ps_qk")
                for kt in range(KT):
                    nc.tensor.matmul(
                        ps_qk,
                        wqk[:, kt, g * 128:(g + 1) * 128],
                        xT[:, kt, csl],
                        start=(kt == 0), stop=(kt == KT - 1))
                qk_post(g, ch, ps_qk)

            def ln_group(g, eng):
                """rstd for group g: transposed ssq matmuls land ms token-on-
                partitions (no reshape DMA); Newton rsqrt on DVE; one scatter
                + two replicating loads on `eng`'s DMA queue."""
                ps_ms = psp.tile([128, 512], F32, tag="misc", bufs=2,
                                 name="ps_mst")
                msv = ps_ms.rearrange("p (c j) -> p c j", j=2)
                for tt in range(NT):
                    nc.tensor.matmul(
                        msv[:, tt, :],
                        sq_g[g][:, tt // 4, (tt % 4) * 128:(tt % 4 + 1) * 128],
                        O2, start=True, stop=True)
                nt_in = wp.tile([128, 32], F32, tag="nt_in", bufs=2)
                nc.vector.tensor_copy(nt_in, ps_ms[:, 0:32])
                t = wp.tile([128, 32], F32, tag="nt_t", bufs=2)
                nc.vector.tensor_scalar(t, nt_in, 1.0 / D, EPS, ALU.mult,
                                        ALU.add)
                y = wp.tile([128, 32], F32, tag="nt_y", bufs=2)
                a = wp.tile([128, 32], F32, tag="nt_a", bufs=2)
                nc.vector.tensor_scalar(y, t, float(_RC[0]), float(_RC[1]),
                                        ALU.mult, ALU.add)
                nc.vector.tensor_mul(y, y, t)
                nc.vector.tensor_scalar_add(y, y, float(_RC[2]))
                nc.vector.tensor_mul(y, y, t)
                nc.vector.tensor_scalar_add(y, y, float(_RC[3]))
                nc.vector.tensor_scalar(y, y, 0.1, 2.4, ALU.max, ALU.min)
                nt_out = wp.tile([128, 2, 16], BF16, tag="nt_out", bufs=2)
                for it in range(2):
                    nc.vector.tensor_mul(a, y, y)
                    nc.vector.tensor_mul(a, a, t)
                    nc.vector.tensor_scalar(a, a, -0.5, 1.5, ALU.mult, ALU.add)
                    if it < 1:
                        nc.vector.tensor_mul(y, y, a)
                    else:
                        nc.vector.tensor_mul(
                            nt_out.rearrange("p j c -> p c j"),
                            y.rearrange("p (c j) -> p c j", j=2),
                            a.rearrange("p (c j) -> p c j", j=2))
                # transpose on PE so both DMA hops stay contiguous:
                # rows_sb[(j,c), tok] -> DRAM rows -> 64-way replicated load
                ps_tr = psp.tile([128, 512], F32, tag="misc", bufs=2,
                                 name="ps_tr")
                nc.tensor.transpose(ps_tr.bitcast(BF16)[0:32, 0:128],
                                    nt_out.rearrange("p j c -> p (j c)"),
                                    eye128)
                rows_sb = wp.tile([32, 128], BF16, tag="rows", bufs=2)
                nc.vector.tensor_copy(rows_sb,
                                      ps_tr.bitcast(BF16)[0:32, 0:128])
                eng.dma_start(out=rstd_g[g].rearrange("j c p -> (j c) p"),
                              in_=rows_sb)
                for j in range(2):
                    row = rstd_g[g][j:j + 1, :, :].rearrange("j c p -> j (c p)")
                    bc = bass.AP(tensor=row.tensor, offset=row.offset,
                                 ap=[[0, 64]] + list(row.ap[1:]))
                    eng.dma_start(out=rb_g[g][64 * j:64 * (j + 1), :], in_=bc)

            def apply_chunk(g, ch):
                """multiply qkt chunk by its per-token rstd broadcast."""
                csl = slice(ch * 512, (ch + 1) * 512)
                nc.vector.tensor_mul(qkt[0:64, g, csl], qkt[0:64, g, csl],
                                     rb_g[g][0:64, csl])
                nc.vector.tensor_mul(qkt[64:128, g, csl], qkt[64:128, g, csl],
                                     rb_g[g][64:128, csl])
                is_q = (g % 2 == 0)
                gcol = None
                if is_q and has_qgamma:
                    gcol = gamma_c[:, 0:1]
                elif not is_q and has_kgamma:
                    gcol = gamma_c[:, 1:2]
                bcol = None
                if is_q and has_qbeta:
                    bcol = beta_c[:, 0:1]
                elif not is_q and has_kbeta:
                    bcol = beta_c[:, 1:2]
                if gcol is not None:
                    nc.vector.tensor_scalar_mul(qkt[:, g, csl],
                                                qkt[:, g, csl], gcol)
                if bcol is not None:
                    nc.vector.tensor_scalar_add(qkt[:, g, csl],
                                                qkt[:, g, csl], bcol)

            def v_feats(tt):
                """v token-major projection for token tile tt."""
                tsl = slice(tt * 128, (tt + 1) * 128)
                ps_v = psp.tile([128, 512], F32, tag="misc", bufs=2,
                                name="ps_v")
                for kt in range(KT):
                    nc.tensor.matmul(
                        ps_v[:, 0:HF], xT[:, kt, tsl], wv[:, kt, :],
                        start=(kt == 0),
                        stop=(not has_vbias and kt == KT - 1))
                if has_vbias:
                    nc.tensor.matmul(ps_v[:, 0:HF], ones512[:, 0:128],
                                     bvT, start=False, stop=True)
                nc.vector.tensor_copy(
                    v_sb[:, tt, :, 0:64],
                    ps_v[:, 0:HF].rearrange("p (h d) -> p h d", h=HPC))

            def normalize(pgq, pqc, pouts):
                """divide PV psum by the fused denominator row, write outT.
                (reciprocal_approx_fast misreads PSUM sources - stage the
                denominator row through SBUF first)"""
                qsl = slice(pqc * 512, (pqc + 1) * 512)
                for hp in range(2):
                    p0 = hp * 64
                    ps_o = pouts[hp]
                    den = wp.tile([1, 512], F32, tag="den", bufs=3)
                    nc.vector.tensor_copy(den, ps_o[64:65, :])
                    rec = wp.tile([1, 512], F32, tag="rec", bufs=3)
                    nc.vector.reciprocal_approx_fast(rec, den)
                    rb2 = wp.tile([64, 512], F32, tag="rb2", bufs=3)
                    nc.gpsimd.partition_broadcast(rb2, rec)
                    nc.vector.tensor_mul(outT_n[p0:p0 + 64, pgq, qsl],
                                         ps_o[0:64, :], rb2)

            def proj_tile(tt):
                """output projection for one token tile (two 512-wide halves
                of DIM in one score-tag psum tile)."""
                tsl = slice(tt * 128, (tt + 1) * 128)
                ps_p = psp.tile([128, 2, 512], F32, tag="score", bufs=2,
                                name="ps_p")
                for fn in range(2):
                    fsl = slice(fn * 512, (fn + 1) * 512)
                    for t in range(2):
                        nc.tensor.matmul(ps_p[:, fn, :],
                                         outT_n[:, t, tsl],
                                         wpj[:, t, fsl],
                                         start=(t == 0), stop=(t == 1))
                ostg = wp.tile([128, DIM], BF16, tag="ostg", bufs=3)
                nc.vector.tensor_copy(ostg, ps_p.rearrange("p a b -> p (a b)"))
                nc.sync.dma_start(out=out_d.ap()[tsl, :], in_=ostg)

            def unit(gq, qc, prev, fillers=None, shift=6, trail=False):
                """scores+exp for unit (gq, qc). PV of `prev` rides `shift`
                slots behind; `fillers[kt]` emits extra PE work at slot kt;
                with trail=True this unit's own PV rides 1 slot behind on
                misc-tag psum (last unit only)."""
                qg, kg = (0, 1) if gq == 0 else (2, 3)
                qsl = slice(qc * 512, (qc + 1) * 512)
                exp_pair = wp.tile([128, NT, 2, 512], BF16, tag="exp",
                                   bufs=2, name="exp_pair")
                pouts = touts = None
                if prev is not None:
                    pgq, pqc, pexp = prev
                    pouts = [psp.tile([65, 512], F32, tag="pvc", bufs=2,
                                      name=f"ps_o{hp}") for hp in range(2)]
                if trail:
                    touts = [psp.tile([65, 512], F32, tag="pvc", bufs=2,
                                      name=f"ps_t{hp}") for hp in range(2)]

                def pv_kt(kt):
                    for hp in range(2):
                        nc.tensor.matmul(
                            pouts[hp], v_sb[:, kt, 2 * pgq + hp, 0:65],
                            pexp[:, kt, hp, :],
                            start=(kt == 0), stop=(kt == NT - 1))

                def pvt_kt(kt):
                    for hp in range(2):
                        nc.tensor.matmul(
                            touts[hp], v_sb[:, kt, 2 * gq + hp, 0:65],
                            exp_pair[:, kt, hp, :],
                            start=(kt == 0), stop=(kt == NT - 1))

                for kt in range(NT):
                    ktsl = slice(kt * 128, (kt + 1) * 128)
                    ps_s = psp.tile([128, 2, 512], F32, tag="score",
                                    bufs=2, name="ps_s")
                    for hp in range(2):
                        p0 = hp * 64
                        nc.tensor.matmul(ps_s[:, hp, :],
                                         qkt[p0:p0 + 64, kg, ktsl],
                                         qkt[p0:p0 + 64, qg, qsl],
                                         start=True, stop=True)
                    nc.scalar.activation(exp_pair[:, kt, :, :], ps_s,
                                         AF.Exp, scale=SCALE)
                    if prev is not None:
                        if trail:
                            # dense handoff: prev PV two pairs per slot at
                            # slots 2..9, freeing pvc mid-unit for our own
                            if 2 <= kt <= 9:
                                pv_kt(2 * (kt - 2))
                                pv_kt(2 * (kt - 2) + 1)
                            if kt == 10:
                                normalize(pgq, pqc, pouts)
                        elif kt >= shift:
                            pv_kt(kt - shift)
                    if trail and 11 <= kt:
                        pvt_kt(2 * (kt - 11))
                        pvt_kt(2 * (kt - 11) + 1)
                    if fillers is not None and kt in fillers:
                        fillers[kt]()
                if prev is not None and not trail:
                    for kt in range(NT - shift, NT):
                        pv_kt(kt)
                    normalize(pgq, pqc, pouts)
                if trail:
                    for kt in range(10, NT):
                        pvt_kt(kt)
                    normalize(gq, qc, touts)
                return (gq, qc, exp_pair)

            # ---- emission (priority order = emission order) ----
            with nc.named_scope("prologue"):
                for g in (1, 0):                      # k01 then q01
                    for cp in range(NCH // 2):
                        qk_chunk_pair(g, cp)
                    ln_group(g, nc.scalar)            # DMA on idle ACT queue
                for ch in range(NCH):
                    apply_chunk(1, ch)
                    apply_chunk(0, ch)

            with nc.named_scope("attn"):
                u = unit(0, 0, None,
                         fillers={kt: (lambda kt=kt: v_feats(kt))
                                  for kt in range(NT)})
                f01 = {1: lambda: qk_chunk(2, 0), 4: lambda: qk_chunk(2, 1),
                       7: lambda: qk_chunk(2, 2), 10: lambda: qk_chunk(2, 3),
                       13: lambda: ln_group(2, nc.gpsimd)}
                u = unit(0, 1, u, fillers=f01)
                f02 = {2: lambda: qk_chunk(3, 0), 6: lambda: qk_chunk(3, 1),
                       9: lambda: qk_chunk(3, 2), 12: lambda: qk_chunk(3, 3),
                       14: lambda: [apply_chunk(2, c) for c in range(NCH)]}
                u = unit(0, 2, u, fillers=f02)
                f03 = {1: lambda: ln_group(3, nc.gpsimd),
                       9: lambda: [apply_chunk(3, c) for c in range(NCH)]}
                u = unit(0, 3, u, fillers=f03)
                u = unit(1, 0, u)
                u = unit(1, 1, u)
                fp0 = {2: lambda: proj_tile(0), 6: lambda: proj_tile(1),
                       10: lambda: proj_tile(2), 14: lambda: proj_tile(3)}
                u = unit(1, 2, u, fillers=fp0)
                fp1 = {0: lambda: proj_tile(4), 1: lambda: proj_tile(5),
                       10: lambda: proj_tile(6), 14: lambda: proj_tile(7)}
                u = unit(1, 3, u, fillers=fp1, trail=True)
            with nc.named_scope("tail"):
                for tt in range(8, 16):
                    proj_tile(tt)

    nc.compile()
    _BUILD_CACHE[key] = nc
    return nc


def _bf16(a):
    return np.ascontiguousarray(a).astype(ml_dtypes.bfloat16)


def kernel(**inputs):
    global LAST_EXEC_NS
    x = np.asarray(inputs["x"], np.float32)
    w_qkv = np.asarray(inputs["w_qkv"], np.float32)
    b_qkv = np.asarray(inputs["b_qkv"], np.float32)
    q_gamma = np.asarray(inputs["q_gamma"], np.float32)
    q_beta = np.asarray(inputs["q_beta"], np.float32)
    k_gamma = np.asarray(inputs["k_gamma"], np.float32)
    k_beta = np.asarray(inputs["k_beta"], np.float32)
    w_proj = np.asarray(inputs["w_proj"], np.float32)
    b_proj = np.asarray(inputs["b_proj"], np.float32)

    has_qkbias = bool(np.any(b_qkv[0:2 * DIM] != 0.0))
    has_qgamma = not bool(np.all(q_gamma == 1.0))
    has_kgamma = not bool(np.all(k_gamma == 1.0))
    has_qbeta = bool(np.any(q_beta != 0.0))
    has_kbeta = bool(np.any(k_beta != 0.0))
    has_vbias = bool(np.any(b_qkv[2 * DIM:3 * DIM] != 0.0))
    nc = _build(has_qkbias, has_qgamma, has_kgamma, has_qbeta, has_kbeta,
                has_vbias)

    # shared constants
    Cd = np.eye(D, dtype=np.float32) - 1.0 / D   # centering matrix (folded
    eye128 = np.eye(128, dtype=np.float32)       # into the qkv weights below)
    gamma_cols = np.stack([np.tile(q_gamma, 2), np.tile(k_gamma, 2)],
                          axis=1).astype(np.float32)
    ones512 = np.ones((1, 512), np.float32)
    beta_cols = np.stack([np.tile(q_beta, 2), np.tile(k_beta, 2)],
                         axis=1).astype(np.float32)
    # pre-center the q/k projection weights and biases per head:
    # LN(Wx+b) centering is linear, so fold (I - J/64) into W and b
    w_qkv = w_qkv.copy()
    b_qkv = b_qkv.copy()
    for h in range(2 * H):            # 16 q heads then 16 k heads
        rs = slice(h * D, (h + 1) * D)
        w_qkv[rs] = Cd @ w_qkv[rs]
        b_qkv[rs] = Cd @ b_qkv[rs]

    in_maps = []
    for c in range(N_CORES):
        b, hg = divmod(c, 4)
        rows = slice(hg * HF, (hg + 1) * HF)
        q_l = w_qkv[0 * DIM:1 * DIM][rows]           # [256, 1024]
        k_l = w_qkv[1 * DIM:2 * DIM][rows]
        v_l = w_qkv[2 * DIM:3 * DIM][rows]
        bq_l = b_qkv[0 * DIM:1 * DIM][rows]
        bk_l = b_qkv[1 * DIM:2 * DIM][rows]
        bv_l = b_qkv[2 * DIM:3 * DIM][rows]
        # group order [q01 | k01 | q23 | k23]
        wqk_rows = np.concatenate([q_l[:128], k_l[:128],
                                   q_l[128:], k_l[128:]], 0)
        bqk_cols = np.stack([bq_l[:128], bk_l[:128], bq_l[128:], bk_l[128:]],
                            axis=1).astype(np.float32)
        def _pm(a, k):
            # [k*128, c] -> partition-major [128, k, c]
            return _bf16(np.ascontiguousarray(
                a.reshape(k, 128, a.shape[1]).transpose(1, 0, 2)))
        m = {
            "xT": _pm(x[b].T, KT),                        # [128, 8, 2048]
            "wqkT": _pm(wqk_rows.T, KT),                  # [128, 8, 512]
            "wvT": _pm(v_l.T, KT),                        # [128, 8, 256]
            "wpT": _pm(w_proj[:, rows].T, 2),             # [128, 2, 1024]
            "eye128": _bf16(eye128),
        }
        if has_qkbias:
            m["bqk_cols"] = bqk_cols
        if has_vbias:
            m["bvT"] = _bf16(bv_l[None, :])
            m["ones512"] = _bf16(ones512)
        if has_qgamma or has_kgamma:
            m["gamma_cols"] = gamma_cols
        if has_qbeta or has_kbeta:
            m["beta_cols"] = beta_cols
        in_maps.append(m)

    res = run_bass_kernel_spmd(nc, in_maps, core_ids=list(range(N_CORES)),
                               trace=TRACE)
    LAST_EXEC_NS = res.exec_time_ns
    globals()["LAST_RESULTS"] = res

    out = np.zeros((B, N, DIM), np.float32)
    for c in range(N_CORES):
        out[c // 4] += np.asarray(res.results[c]["out_partial"], np.float32)
    out += b_proj[None, None, :]
    return out


# revision 19
# speedup vs baseline: 1.0449x; 1.0182x over previous
"""Multi-head attention (qk-norm variant) on 8 TRN2 NeuronCores.

Sharding (Megatron-style, per spec hint): core c handles batch b=c//4 and
head-group hg=c%4 (4 of 16 heads). QKV is column-parallel, attention is fully
local per (b, head), and the output projection is row-parallel: each core
produces a partial [N, DIM] output which the host sums per batch and adds
b_proj.

Per-core kernel (bf16 compute, fp32 PSUM accumulation). v5 design:
  - The ACT engine's exp stream (128 calls x ~1.15us over the [128,2,512]
    score tiles) is the roofline; everything else is scheduled to feed it
    without gaps:
      * per key-tile emission slots: scores pair -> exp -> PV of the previous
        unit (shifted 6 slots so the previous normalize can free its psum)
        -> one slice of "filler" PE work (v projection, qkv for heads 2/3,
        output projection), sized to keep the per-slot PE time under the
        1.15us exp cadence so the tensor engine stays busy (HAM stays warm)
        but never starves ACT.
  - qk-norm rstd has NO scalar-engine work and NO serial DMA reshape chain:
    sum-of-squares comes from a TRANSPOSED ones matmul (lhsT = squares tile,
    rhs = O2) that lands token-on-partitions [128,(kt,head)] directly; rstd =
    1/sqrt(ms/64+eps) via cubic seed + 3 Newton steps on DVE; the only DMA is
    one scatter to DRAM + one partition-replicating load back (64-way
    broadcast), issued on the ACT queue for the prologue groups (idle before
    the exp stream) and on the gpsimd queue for heads 2/3 (so the Sync and
    ACT queues never head-of-line block on the LN chain).
  - weights are pre-centered on the host (LN mean subtraction is linear);
    softmax needs no max-subtraction (|logits| <= 8 after qk-norm).
  - v is token-major with a fused ones column: the PV matmul (M=65) yields
    the softmax denominator as psum row 64 for free.
"""
import numpy as np
import ml_dtypes

import concourse.bass as bass
import concourse.bacc as bacc
import concourse.tile as tile
from concourse import mybir
from concourse.bass_utils import run_bass_kernel_spmd

F32 = mybir.dt.float32
BF16 = mybir.dt.bfloat16
AF = mybir.ActivationFunctionType
ALU = mybir.AluOpType

B, N, DIM = 2, 2048, 1024
H, D = 16, 64
EPS = 1e-5
N_CORES = 8
HPC = 4              # heads per core
HF = HPC * D         # 256 local head features
KT = DIM // 128      # 8 contraction tiles
NT = N // 128        # 16 token tiles
NCH = N // 512       # 4 token chunks
SCALE = D ** -0.5

# rsqrt seed polynomial (relative-error weighted cubic fit on [0.2, 3.5];
# with 3 Newton steps max rel err < 5e-5 on the realistic ms range)
_tt = np.linspace(0.2, 3.5, 4000)
_RC = np.polyfit(_tt, _tt ** -0.5, 3, w=_tt ** 0.5)

# set by test harness to request NTFF profiling
TRACE = False
LAST_EXEC_NS = None
LAST_RESULTS = None

_BUILD_CACHE = {}


def _build(has_qkbias, has_qgamma, has_kgamma, has_qbeta, has_kbeta,
           has_vbias):
    key = (has_qkbias, has_qgamma, has_kgamma, has_qbeta, has_kbeta,
           has_vbias)
    if key in _BUILD_CACHE:
        return _BUILD_CACHE[key]

    nc = bacc.Bacc("TRN2", target_bir_lowering=False, debug=False,
                   num_devices=N_CORES)

    # all inputs arrive partition-major so each DMA is one fat descriptor
    # with long contiguous runs per partition
    xT_d = nc.dram_tensor("xT", [128, KT, N], BF16, kind="ExternalInput")
    # columns ordered [q01 | k01 | q23 | k23] so group g = cols g*128..
    wqkT_d = nc.dram_tensor("wqkT", [128, KT, 2 * HF], BF16,
                            kind="ExternalInput")
    wvT_d = nc.dram_tensor("wvT", [128, KT, HF], BF16, kind="ExternalInput")
    wpT_d = nc.dram_tensor("wpT", [128, 2, DIM], BF16, kind="ExternalInput")
    eye_d = nc.dram_tensor("eye128", [128, 128], BF16, kind="ExternalInput")
    bqk_d = ones_d = bvT_d = gamma_d = beta_d = None
    if has_qkbias:
        bqk_d = nc.dram_tensor("bqk_cols", [128, 4], F32, kind="ExternalInput")
    if has_vbias:
        bvT_d = nc.dram_tensor("bvT", [1, HF], BF16, kind="ExternalInput")
        ones_d = nc.dram_tensor("ones512", [1, 512], BF16, kind="ExternalInput")
    if has_qgamma or has_kgamma:
        gamma_d = nc.dram_tensor("gamma_cols", [128, 2], F32, kind="ExternalInput")
    if has_qbeta or has_kbeta:
        beta_d = nc.dram_tensor("beta_cols", [128, 2], F32, kind="ExternalInput")
    out_d = nc.dram_tensor("out_partial", [N, DIM], BF16, kind="ExternalOutput")

    with tile.TileContext(nc) as tc:
        with (
            tc.tile_pool(name="persist", bufs=1) as pp,
            tc.tile_pool(name="work", bufs=2) as wp,
            tc.tile_pool(name="psum", bufs=1, space="PSUM") as psp,
            tc.tile_pool(name="dram", bufs=1, space="DRAM") as dp,
        ):
            # ---- persistent SBUF tensors ----
            xT = pp.tile([128, KT, N], BF16)
            wqk = pp.tile([128, KT, 2 * HF], BF16)
            wv = pp.tile([128, KT, HF], BF16)
            wpj = pp.tile([128, 2, DIM], BF16)
            O2 = pp.tile([128, 2], BF16)
            eye128 = pp.tile([128, 128], BF16)
            bqk = pp.tile([128, 4], F32) if bqk_d is not None else None
            bvT = pp.tile([1, HF], BF16) if bvT_d is not None else None
            ones512 = pp.tile([1, 512], BF16) if ones_d is not None else None
            gamma_c = pp.tile([128, 2], F32) if gamma_d is not None else None
            beta_c = pp.tile([128, 2], F32) if beta_d is not None else None

            # v token-major with a ones column at index 64
            v_sb = pp.tile([128, NT, HPC, 66], BF16)
            # q/k head-major, groups g: 0=q01 1=k01 2=q23 3=k23
            qkt = pp.tile([128, 4, N], BF16)
            outT_n = pp.tile([128, 2, N], BF16)   # attn out, head-major

            # rstd DRAM bounce target, [head j, kt, token-in-tile] per group
            rstd_g = [dp.tile([2, 16, 128], BF16, name=f"rstd{g}")
                      for g in range(4)]

            # ---- input DMA: few big descriptors, parallel rings ----
            xv = xT_d.ap()
            nc.sync.dma_start(out=xT[:, 0:4, :], in_=xv[:, 0:4, :])
            nc.sync.dma_start(out=xT[:, 4:8, :], in_=xv[:, 4:8, :])
            nc.sync.dma_start(out=wqk, in_=wqkT_d.ap())
            nc.sync.dma_start(out=wv, in_=wvT_d.ap())
            nc.sync.dma_start(out=wpj, in_=wpT_d.ap())
            nc.sync.dma_start(out=eye128, in_=eye_d.ap())
            for t, d in [(bqk, bqk_d), (bvT, bvT_d), (ones512, ones_d),
                         (gamma_c, gamma_d), (beta_c, beta_d)]:
                if t is not None:
                    nc.sync.dma_start(out=t, in_=d.ap())

            nc.vector.memset(v_sb[:, :, :, 64:66], 0.0)
            nc.vector.memset(v_sb[:, :, :, 64:65], 1.0)
            nc.vector.memset(O2[0:64, 0:1], 1.0)
            nc.vector.memset(O2[64:128, 0:1], 0.0)
            nc.vector.memset(O2[0:64, 1:2], 0.0)
            nc.vector.memset(O2[64:128, 1:2], 1.0)

            sq_g = {}
            rb_g = {}

            def _g_tiles(g):
                if g not in sq_g:
                    sq_g[g] = wp.tile([128, NCH, 512], BF16, tag="sq", bufs=2,
                                      name=f"sq{g}")
                    rb_g[g] = wp.tile([128, N], BF16, tag="rb", bufs=2,
                                      name=f"rb{g}")

            def qk_post(g, ch, ps):
                """PSUM -> qkt copy and square for one finished chunk."""
                csl = slice(ch * 512, (ch + 1) * 512)
                if has_qkbias:
                    nc.vector.tensor_scalar_add(
                        qkt[:, g, csl], ps, bqk[:, g:g + 1])
                else:
                    nc.vector.tensor_copy(qkt[:, g, csl], ps)
                nc.vector.tensor_mul(sq_g[g][:, ch, :], qkt[:, g, csl],
                                     qkt[:, g, csl])

            def qk_chunk_pair(g, cp):
                """two chunks (2cp, 2cp+1) of group g in one score-tag psum
                tile (prologue; shared lhsT per kt amortizes weight loads)."""
                _g_tiles(g)
                ps2 = psp.tile([128, 2, 512], F32, tag="score", bufs=2,
                               name="ps_qk2")
                for kt in range(KT):
                    for i in range(2):
                        csl = slice((2 * cp + i) * 512, (2 * cp + i + 1) * 512)
                        nc.tensor.matmul(
                            ps2[:, i, :],
                            wqk[:, kt, g * 128:(g + 1) * 128],
                            xT[:, kt, csl],
                            start=(kt == 0), stop=(kt == KT - 1))
                for i in range(2):
                    qk_post(g, 2 * cp + i, ps2[:, i, :])

            def qk_chunk(g, ch):
                """single-chunk qkv on the misc tag (mid-stream filler)."""
                _g_tiles(g)
                csl = slice(ch * 512, (ch + 1) * 512)
                ps_qk = psp.tile([128, 512], F32, tag="misc", bufs=2,
                                 name="ps_qk")
                for kt in range(KT):
                    nc.tensor.matmul(
                        ps_qk,
                        wqk[:, kt, g * 128:(g + 1) * 128],
                        xT[:, kt, csl],
                        start=(kt == 0), stop=(kt == KT - 1))
                qk_post(g, ch, ps_qk)

            def ln_group(g, eng):
                """rstd for group g: transposed ssq matmuls land ms token-on-
                partitions (no reshape DMA); Newton rsqrt on DVE; one scatter
                + two replicating loads on `eng`'s DMA queue."""
                ps_ms = psp.tile([128, 512], F32, tag="misc", bufs=2,
                                 name="ps_mst")
                msv = ps_ms.rearrange("p (c j) -> p c j", j=2)
                for tt in range(NT):
                    nc.tensor.matmul(
                        msv[:, tt, :],
                        sq_g[g][:, tt // 4, (tt % 4) * 128:(tt % 4 + 1) * 128],
                        O2, start=True, stop=True)
                nt_in = wp.tile([128, 32], F32, tag="nt_in", bufs=2)
                nc.vector.tensor_copy(nt_in, ps_ms[:, 0:32])
                t = wp.tile([128, 32], F32, tag="nt_t", bufs=2)
                nc.vector.tensor_scalar(t, nt_in, 1.0 / D, EPS, ALU.mult,
                                        ALU.add)
                y = wp.tile([128, 32], F32, tag="nt_y", bufs=2)
                a = wp.tile([128, 32], F32, tag="nt_a", bufs=2)
                nc.vector.tensor_scalar(y, t, float(_RC[0]), float(_RC[1]),
                                        ALU.mult, ALU.add)
                nc.vector.tensor_mul(y, y, t)
                nc.vector.tensor_scalar_add(y, y, float(_RC[2]))
                nc.vector.tensor_mul(y, y, t)
                nc.vector.tensor_scalar_add(y, y, float(_RC[3]))
                nc.vector.tensor_scalar(y, y, 0.1, 2.4, ALU.max, ALU.min)
                nt_out = wp.tile([128, 2, 16], BF16, tag="nt_out", bufs=2)
                for it in range(2):
                    nc.vector.tensor_mul(a, y, y)
                    nc.vector.tensor_mul(a, a, t)
                    nc.vector.tensor_scalar(a, a, -0.5, 1.5, ALU.mult, ALU.add)
                    if it < 1:
                        nc.vector.tensor_mul(y, y, a)
                    else:
                        nc.vector.tensor_mul(
                            nt_out.rearrange("p j c -> p c j"),
                            y.rearrange("p (c j) -> p c j", j=2),
                            a.rearrange("p (c j) -> p c j", j=2))
                # transpose on PE so both DMA hops stay contiguous:
                # rows_sb[(j,c), tok] -> DRAM rows -> 64-way replicated load
                ps_tr = psp.tile([128, 512], F32, tag="misc", bufs=2,
                                 name="ps_tr")
                nc.tensor.transpose(ps_tr.bitcast(BF16)[0:32, 0:128],
                                    nt_out.rearrange("p j c -> p (j c)"),
                                    eye128)
                rows_sb = wp.tile([32, 128], BF16, tag="rows", bufs=2)
                nc.vector.tensor_copy(rows_sb,
                                      ps_tr.bitcast(BF16)[0:32, 0:128])
                eng.dma_start(out=rstd_g[g].rearrange("j c p -> (j c) p"),
                              in_=rows_sb)
                for j in range(2):
                    row = rstd_g[g][j:j + 1, :, :].rearrange("j c p -> j (c p)")
                    bc = bass.AP(tensor=row.tensor, offset=row.offset,
                                 ap=[[0, 64]] + list(row.ap[1:]))
                    eng.dma_start(out=rb_g[g][64 * j:64 * (j + 1), :], in_=bc)

            def apply_chunk(g, ch):
                """multiply qkt chunk by its per-token rstd broadcast."""
                csl = slice(ch * 512, (ch + 1) * 512)
                nc.vector.tensor_mul(qkt[0:64, g, csl], qkt[0:64, g, csl],
                                     rb_g[g][0:64, csl])
                nc.vector.tensor_mul(qkt[64:128, g, csl], qkt[64:128, g, csl],
                                     rb_g[g][64:128, csl])
                is_q = (g % 2 == 0)
                gcol = None
                if is_q and has_qgamma:
                    gcol = gamma_c[:, 0:1]
                elif not is_q and has_kgamma:
                    gcol = gamma_c[:, 1:2]
                bcol = None
                if is_q and has_qbeta:
                    bcol = beta_c[:, 0:1]
                elif not is_q and has_kbeta:
                    bcol = beta_c[:, 1:2]
                if gcol is not None:
                    nc.vector.tensor_scalar_mul(qkt[:, g, csl],
                                                qkt[:, g, csl], gcol)
                if bcol is not None:
                    nc.vector.tensor_scalar_add(qkt[:, g, csl],
                                                qkt[:, g, csl], bcol)

            def v_feats(tt):
                """v token-major projection for token tile tt."""
                tsl = slice(tt * 128, (tt + 1) * 128)
                ps_v = psp.tile([128, 512], F32, tag="misc", bufs=2,
                                name="ps_v")
                for kt in range(KT):
                    nc.tensor.matmul(
                        ps_v[:, 0:HF], xT[:, kt, tsl], wv[:, kt, :],
                        start=(kt == 0),
                        stop=(not has_vbias and kt == KT - 1))
                if has_vbias:
                    nc.tensor.matmul(ps_v[:, 0:HF], ones512[:, 0:128],
                                     bvT, start=False, stop=True)
                nc.vector.tensor_copy(
                    v_sb[:, tt, :, 0:64],
                    ps_v[:, 0:HF].rearrange("p (h d) -> p h d", h=HPC))

            def normalize(pgq, pqc, pouts):
                """divide PV psum by the fused denominator row, write outT.
                (reciprocal_approx_fast misreads PSUM sources - stage the
                denominator row through SBUF first)"""
                qsl = slice(pqc * 512, (pqc + 1) * 512)
                for hp in range(2):
                    p0 = hp * 64
                    ps_o = pouts[hp]
                    den = wp.tile([1, 512], F32, tag="den", bufs=3)
                    nc.vector.tensor_copy(den, ps_o[64:65, :])
                    rec = wp.tile([1, 512], F32, tag="rec", bufs=3)
                    nc.vector.reciprocal_approx_fast(rec, den)
                    rb2 = wp.tile([64, 512], F32, tag="rb2", bufs=3)
                    nc.gpsimd.partition_broadcast(rb2, rec)
                    nc.vector.tensor_mul(outT_n[p0:p0 + 64, pgq, qsl],
                                         ps_o[0:64, :], rb2)

            def proj_tile(tt):
                """output projection for one token tile (two 512-wide halves
                of DIM in one score-tag psum tile)."""
                tsl = slice(tt * 128, (tt + 1) * 128)
                ostg = wp.tile([128, DIM], BF16, tag="ostg", bufs=3)
                for fn in range(2):
                    fsl = slice(fn * 512, (fn + 1) * 512)
                    ps_p = psp.tile([128, 512], F32, tag="misc", bufs=2,
                                    name="ps_p")
                    for t in range(2):
                        nc.tensor.matmul(ps_p,
                                         outT_n[:, t, tsl],
                                         wpj[:, t, fsl],
                                         start=(t == 0), stop=(t == 1))
                    nc.vector.tensor_copy(ostg[:, fsl], ps_p)
                nc.sync.dma_start(out=out_d.ap()[tsl, :], in_=ostg)

            def unit(gq, qc, prev, fillers=None, shift=6, trail=False):
                """scores+exp for unit (gq, qc). PV of `prev` rides `shift`
                slots behind; `fillers[kt]` emits extra PE work at slot kt;
                with trail=True this unit's own PV rides 1 slot behind on
                misc-tag psum (last unit only)."""
                qg, kg = (0, 1) if gq == 0 else (2, 3)
                qsl = slice(qc * 512, (qc + 1) * 512)
                exp_pair = wp.tile([128, NT, 2, 512], BF16, tag="exp",
                                   bufs=2, name="exp_pair")
                pouts = touts = None
                if prev is not None:
                    pgq, pqc, pexp = prev
                    pouts = [psp.tile([65, 512], F32, tag="pvc", bufs=2,
                                      name=f"ps_o{hp}") for hp in range(2)]
                if trail:
                    touts = [psp.tile([65, 512], F32, tag="pvc", bufs=2,
                                      name=f"ps_t{hp}") for hp in range(2)]

                def pv_kt(kt):
                    for hp in range(2):
                        nc.tensor.matmul(
                            pouts[hp], v_sb[:, kt, 2 * pgq + hp, 0:65],
                            pexp[:, kt, hp, :],
                            start=(kt == 0), stop=(kt == NT - 1))

                def pvt_kt(kt):
                    for hp in range(2):
                        nc.tensor.matmul(
                            touts[hp], v_sb[:, kt, 2 * gq + hp, 0:65],
                            exp_pair[:, kt, hp, :],
                            start=(kt == 0), stop=(kt == NT - 1))

                for kt in range(NT):
                    ktsl = slice(kt * 128, (kt + 1) * 128)
                    ps_s = psp.tile([128, 2, 512], F32, tag="score",
                                    bufs=2, name="ps_s")
                    for hp in range(2):
                        p0 = hp * 64
                        nc.tensor.matmul(ps_s[:, hp, :],
                                         qkt[p0:p0 + 64, kg, ktsl],
                                         qkt[p0:p0 + 64, qg, qsl],
                                         start=True, stop=True)
                    nc.scalar.activation(exp_pair[:, kt, :, :], ps_s,
                                         AF.Exp, scale=SCALE)
                    if prev is not None:
                        if trail:
                            # dense handoff: prev PV two pairs per slot at
                            # slots 2..9, freeing pvc mid-unit for our own
                            if 2 <= kt <= 9:
                                pv_kt(2 * (kt - 2))
                                pv_kt(2 * (kt - 2) + 1)
                            if kt == 10:
                                normalize(pgq, pqc, pouts)
                        elif kt >= shift:
                            pv_kt(kt - shift)
                    if trail and 11 <= kt:
                        pvt_kt(2 * (kt - 11))
                        pvt_kt(2 * (kt - 11) + 1)
                    if fillers is not None and kt in fillers:
                        fillers[kt]()
                if prev is not None and not trail:
                    for kt in range(NT - shift, NT):
                        pv_kt(kt)
                    normalize(pgq, pqc, pouts)
                if trail:
                    for kt in range(10, NT):
                        pvt_kt(kt)
                    normalize(gq, qc, touts)
                return (gq, qc, exp_pair)

            # ---- emission (priority order = emission order) ----
            with nc.named_scope("prologue"):
                for g in (1, 0):                      # k01 then q01
                    for cp in range(NCH // 2):
                        qk_chunk_pair(g, cp)
                    ln_group(g, nc.scalar)            # DMA on idle ACT queue
                for ch in range(NCH):
                    apply_chunk(1, ch)
                    apply_chunk(0, ch)

            with nc.named_scope("attn"):
                u = unit(0, 0, None,
                         fillers={kt: (lambda kt=kt: v_feats(kt))
                                  for kt in range(NT)})
                f01 = {1: lambda: qk_chunk(2, 0), 4: lambda: qk_chunk(2, 1),
                       7: lambda: qk_chunk(2, 2), 10: lambda: qk_chunk(2, 3)}
                u = unit(0, 1, u, fillers=f01)
                f02 = {0: lambda: ln_group(2, nc.gpsimd),
                       2: lambda: qk_chunk(3, 0), 6: lambda: qk_chunk(3, 1),
                       9: lambda: qk_chunk(3, 2), 12: lambda: qk_chunk(3, 3),
                       14: lambda: [apply_chunk(2, c) for c in range(NCH)]}
                u = unit(0, 2, u, fillers=f02)
                f03 = {1: lambda: ln_group(3, nc.gpsimd),
                       9: lambda: [apply_chunk(3, c) for c in range(NCH)]}
                u = unit(0, 3, u, fillers=f03)
                u = unit(1, 0, u)
                u = unit(1, 1, u)
                fp0 = {2: lambda: proj_tile(0), 6: lambda: proj_tile(1),
                       10: lambda: proj_tile(2), 14: lambda: proj_tile(3)}
                u = unit(1, 2, u, fillers=fp0)
                fp1 = {0: lambda: proj_tile(4), 1: lambda: proj_tile(5),
                       10: lambda: proj_tile(6), 14: lambda: proj_tile(7)}
                u = unit(1, 3, u, fillers=fp1, trail=True)
            with nc.named_scope("tail"):
                for tt in range(8, 16):
                    proj_tile(tt)

    nc.compile()
    _BUILD_CACHE[key] = nc
    return nc


def _bf16(a):
    return np.ascontiguousarray(a).astype(ml_dtypes.bfloat16)


def kernel(**inputs):
    global LAST_EXEC_NS
    x = np.asarray(inputs["x"], np.float32)
    w_qkv = np.asarray(inputs["w_qkv"], np.float32)
    b_qkv = np.asarray(inputs["b_qkv"], np.float32)
    q_gamma = np.asarray(inputs["q_gamma"], np.float32)
    q_beta = np.asarray(inputs["q_beta"], np.float32)
    k_gamma = np.asarray(inputs["k_gamma"], np.float32)
    k_beta = np.asarray(inputs["k_beta"], np.float32)
    w_proj = np.asarray(inputs["w_proj"], np.float32)
    b_proj = np.asarray(inputs["b_proj"], np.float32)

    has_qkbias = bool(np.any(b_qkv[0:2 * DIM] != 0.0))
    has_qgamma = not bool(np.all(q_gamma == 1.0))
    has_kgamma = not bool(np.all(k_gamma == 1.0))
    has_qbeta = bool(np.any(q_beta != 0.0))
    has_kbeta = bool(np.any(k_beta != 0.0))
    has_vbias = bool(np.any(b_qkv[2 * DIM:3 * DIM] != 0.0))
    nc = _build(has_qkbias, has_qgamma, has_kgamma, has_qbeta, has_kbeta,
                has_vbias)

    # shared constants
    Cd = np.eye(D, dtype=np.float32) - 1.0 / D   # centering matrix (folded
    eye128 = np.eye(128, dtype=np.float32)       # into the qkv weights below)
    gamma_cols = np.stack([np.tile(q_gamma, 2), np.tile(k_gamma, 2)],
                          axis=1).astype(np.float32)
    ones512 = np.ones((1, 512), np.float32)
    beta_cols = np.stack([np.tile(q_beta, 2), np.tile(k_beta, 2)],
                         axis=1).astype(np.float32)
    # pre-center the q/k projection weights and biases per head:
    # LN(Wx+b) centering is linear, so fold (I - J/64) into W and b
    w_qkv = w_qkv.copy()
    b_qkv = b_qkv.copy()
    for h in range(2 * H):            # 16 q heads then 16 k heads
        rs = slice(h * D, (h + 1) * D)
        w_qkv[rs] = Cd @ w_qkv[rs]
        b_qkv[rs] = Cd @ b_qkv[rs]

    in_maps = []
    for c in range(N_CORES):
        b, hg = divmod(c, 4)
        rows = slice(hg * HF, (hg + 1) * HF)
        q_l = w_qkv[0 * DIM:1 * DIM][rows]           # [256, 1024]
        k_l = w_qkv[1 * DIM:2 * DIM][rows]
        v_l = w_qkv[2 * DIM:3 * DIM][rows]
        bq_l = b_qkv[0 * DIM:1 * DIM][rows]
        bk_l = b_qkv[1 * DIM:2 * DIM][rows]
        bv_l = b_qkv[2 * DIM:3 * DIM][rows]
        # group order [q01 | k01 | q23 | k23]
        wqk_rows = np.concatenate([q_l[:128], k_l[:128],
                                   q_l[128:], k_l[128:]], 0)
        bqk_cols = np.stack([bq_l[:128], bk_l[:128], bq_l[128:], bk_l[128:]],
                            axis=1).astype(np.float32)
        def _pm(a, k):
            # [k*128, c] -> partition-major [128, k, c]
            return _bf16(np.ascontiguousarray(
                a.reshape(k, 128, a.shape[1]).transpose(1, 0, 2)))
        m = {
            "xT": _pm(x[b].T, KT),                        # [128, 8, 2048]
            "wqkT": _pm(wqk_rows.T, KT),                  # [128, 8, 512]
            "wvT": _pm(v_l.T, KT),                        # [128, 8, 256]
            "wpT": _pm(w_proj[:, rows].T, 2),             # [128, 2, 1024]
            "eye128": _bf16(eye128),
        }
        if has_qkbias:
            m["bqk_cols"] = bqk_cols
        if has_vbias:
            m["bvT"] = _bf16(bv_l[None, :])
            m["ones512"] = _bf16(ones512)
        if has_qgamma or has_kgamma:
            m["gamma_cols"] = gamma_cols
        if has_qbeta or has_kbeta:
            m["beta_cols"] = beta_cols
        in_maps.append(m)

    res = run_bass_kernel_spmd(nc, in_maps, core_ids=list(range(N_CORES)),
                               trace=TRACE)
    LAST_EXEC_NS = res.exec_time_ns
    globals()["LAST_RESULTS"] = res

    out = np.zeros((B, N, DIM), np.float32)
    for c in range(N_CORES):
        out[c // 4] += np.asarray(res.results[c]["out_partial"], np.float32)
    out += b_proj[None, None, :]
    return out


# revision 20
# speedup vs baseline: 1.0535x; 1.0082x over previous
"""Multi-head attention (qk-norm variant) on 8 TRN2 NeuronCores.

Sharding (Megatron-style, per spec hint): core c handles batch b=c//4 and
head-group hg=c%4 (4 of 16 heads). QKV is column-parallel, attention is fully
local per (b, head), and the output projection is row-parallel: each core
produces a partial [N, DIM] output which the host sums per batch and adds
b_proj.

Per-core kernel (bf16 compute, fp32 PSUM accumulation). v5 design:
  - The ACT engine's exp stream (128 calls x ~1.15us over the [128,2,512]
    score tiles) is the roofline; everything else is scheduled to feed it
    without gaps:
      * per key-tile emission slots: scores pair -> exp -> PV of the previous
        unit (shifted 6 slots so the previous normalize can free its psum)
        -> one slice of "filler" PE work (v projection, qkv for heads 2/3,
        output projection), sized to keep the per-slot PE time under the
        1.15us exp cadence so the tensor engine stays busy (HAM stays warm)
        but never starves ACT.
  - qk-norm rstd has NO scalar-engine work and NO serial DMA reshape chain:
    sum-of-squares comes from a TRANSPOSED ones matmul (lhsT = squares tile,
    rhs = O2) that lands token-on-partitions [128,(kt,head)] directly; rstd =
    1/sqrt(ms/64+eps) via cubic seed + 3 Newton steps on DVE; the only DMA is
    one scatter to DRAM + one partition-replicating load back (64-way
    broadcast), issued on the ACT queue for the prologue groups (idle before
    the exp stream) and on the gpsimd queue for heads 2/3 (so the Sync and
    ACT queues never head-of-line block on the LN chain).
  - weights are pre-centered on the host (LN mean subtraction is linear);
    softmax needs no max-subtraction (|logits| <= 8 after qk-norm).
  - v is token-major with a fused ones column: the PV matmul (M=65) yields
    the softmax denominator as psum row 64 for free.
"""
import numpy as np
import ml_dtypes

import concourse.bass as bass
import concourse.bacc as bacc
import concourse.tile as tile
from concourse import mybir
from concourse.bass_utils import run_bass_kernel_spmd

F32 = mybir.dt.float32
BF16 = mybir.dt.bfloat16
AF = mybir.ActivationFunctionType
ALU = mybir.AluOpType

B, N, DIM = 2, 2048, 1024
H, D = 16, 64
EPS = 1e-5
N_CORES = 8
HPC = 4              # heads per core
HF = HPC * D         # 256 local head features
KT = DIM // 128      # 8 contraction tiles
NT = N // 128        # 16 token tiles
NCH = N // 512       # 4 token chunks
SCALE = D ** -0.5

# rsqrt seed polynomial (relative-error weighted cubic fit on [0.2, 3.5];
# with 3 Newton steps max rel err < 5e-5 on the realistic ms range)
_tt = np.linspace(0.2, 3.5, 4000)
_RC = np.polyfit(_tt, _tt ** -0.5, 3, w=_tt ** 0.5)

# set by test harness to request NTFF profiling
TRACE = False
LAST_EXEC_NS = None
LAST_RESULTS = None

_BUILD_CACHE = {}


def _build(has_qkbias, has_qgamma, has_kgamma, has_qbeta, has_kbeta,
           has_vbias):
    key = (has_qkbias, has_qgamma, has_kgamma, has_qbeta, has_kbeta,
           has_vbias)
    if key in _BUILD_CACHE:
        return _BUILD_CACHE[key]

    nc = bacc.Bacc("TRN2", target_bir_lowering=False, debug=False,
                   num_devices=N_CORES)

    # all inputs arrive partition-major so each DMA is one fat descriptor
    # with long contiguous runs per partition
    xT_d = nc.dram_tensor("xT", [128, KT, N], BF16, kind="ExternalInput")
    # columns ordered [q01 | k01 | q23 | k23] so group g = cols g*128..
    wqkT_d = nc.dram_tensor("wqkT", [128, KT, 2 * HF], BF16,
                            kind="ExternalInput")
    wvT_d = nc.dram_tensor("wvT", [128, KT, HF], BF16, kind="ExternalInput")
    wpT_d = nc.dram_tensor("wpT", [128, 2, DIM], BF16, kind="ExternalInput")
    eye_d = nc.dram_tensor("eye128", [128, 128], BF16, kind="ExternalInput")
    bqk_d = ones_d = bvT_d = gamma_d = beta_d = None
    if has_qkbias:
        bqk_d = nc.dram_tensor("bqk_cols", [128, 4], F32, kind="ExternalInput")
    if has_vbias:
        bvT_d = nc.dram_tensor("bvT", [1, HF], BF16, kind="ExternalInput")
        ones_d = nc.dram_tensor("ones512", [1, 512], BF16, kind="ExternalInput")
    if has_qgamma or has_kgamma:
        gamma_d = nc.dram_tensor("gamma_cols", [128, 2], F32, kind="ExternalInput")
    if has_qbeta or has_kbeta:
        beta_d = nc.dram_tensor("beta_cols", [128, 2], F32, kind="ExternalInput")
    out_d = nc.dram_tensor("out_partial", [N, DIM], BF16, kind="ExternalOutput")

    with tile.TileContext(nc) as tc:
        with (
            tc.tile_pool(name="persist", bufs=1) as pp,
            tc.tile_pool(name="work", bufs=2) as wp,
            tc.tile_pool(name="psum", bufs=1, space="PSUM") as psp,
            tc.tile_pool(name="dram", bufs=1, space="DRAM") as dp,
        ):
            # ---- persistent SBUF tensors ----
            xT = pp.tile([128, KT, N], BF16)
            wqk = pp.tile([128, KT, 2 * HF], BF16)
            wv = pp.tile([128, KT, HF], BF16)
            wpj = pp.tile([128, 2, DIM], BF16)
            O2 = pp.tile([128, 2], BF16)
            eye128 = pp.tile([128, 128], BF16)
            bqk = pp.tile([128, 4], F32) if bqk_d is not None else None
            bvT = pp.tile([1, HF], BF16) if bvT_d is not None else None
            ones512 = pp.tile([1, 512], BF16) if ones_d is not None else None
            gamma_c = pp.tile([128, 2], F32) if gamma_d is not None else None
            beta_c = pp.tile([128, 2], F32) if beta_d is not None else None

            # v token-major with a ones column at index 64
            v_sb = pp.tile([128, NT, HPC, 66], BF16)
            # q/k head-major, groups g: 0=q01 1=k01 2=q23 3=k23
            qkt = pp.tile([128, 4, N], BF16)
            outT_n = pp.tile([128, 2, N], BF16)   # attn out, head-major

            # rstd DRAM bounce target, [head j, kt, token-in-tile] per group
            rstd_g = [dp.tile([2, 16, 128], BF16, name=f"rstd{g}")
                      for g in range(4)]

            # ---- input DMA: few big descriptors, parallel rings ----
            xv = xT_d.ap()
            nc.sync.dma_start(out=xT[:, 0:4, :], in_=xv[:, 0:4, :])
            nc.sync.dma_start(out=xT[:, 4:8, :], in_=xv[:, 4:8, :])
            nc.sync.dma_start(out=wqk, in_=wqkT_d.ap())
            nc.sync.dma_start(out=wv, in_=wvT_d.ap())
            nc.sync.dma_start(out=wpj, in_=wpT_d.ap())
            nc.sync.dma_start(out=eye128, in_=eye_d.ap())
            for t, d in [(bqk, bqk_d), (bvT, bvT_d), (ones512, ones_d),
                         (gamma_c, gamma_d), (beta_c, beta_d)]:
                if t is not None:
                    nc.sync.dma_start(out=t, in_=d.ap())

            nc.vector.memset(v_sb[:, :, :, 64:66], 0.0)
            nc.vector.memset(v_sb[:, :, :, 64:65], 1.0)
            nc.vector.memset(O2[0:64, 0:1], 1.0)
            nc.vector.memset(O2[64:128, 0:1], 0.0)
            nc.vector.memset(O2[0:64, 1:2], 0.0)
            nc.vector.memset(O2[64:128, 1:2], 1.0)

            sq_g = {}
            rb_g = {}

            def _g_tiles(g):
                if g not in sq_g:
                    sq_g[g] = wp.tile([128, NCH, 512], BF16, tag="sq", bufs=2,
                                      name=f"sq{g}")
                    rb_g[g] = wp.tile([128, N], BF16, tag="rb", bufs=2,
                                      name=f"rb{g}")

            def qk_post(g, ch, ps):
                """PSUM -> qkt copy and square for one finished chunk."""
                csl = slice(ch * 512, (ch + 1) * 512)
                if has_qkbias:
                    nc.vector.tensor_scalar_add(
                        qkt[:, g, csl], ps, bqk[:, g:g + 1])
                else:
                    nc.vector.tensor_copy(qkt[:, g, csl], ps)
                nc.vector.tensor_mul(sq_g[g][:, ch, :], qkt[:, g, csl],
                                     qkt[:, g, csl])

            def qk_chunk_pair(g, cp):
                """two chunks (2cp, 2cp+1) of group g in one score-tag psum
                tile (prologue; shared lhsT per kt amortizes weight loads)."""
                _g_tiles(g)
                ps2 = psp.tile([128, 2, 512], F32, tag="score", bufs=2,
                               name="ps_qk2")
                for kt in range(KT):
                    for i in range(2):
                        csl = slice((2 * cp + i) * 512, (2 * cp + i + 1) * 512)
                        nc.tensor.matmul(
                            ps2[:, i, :],
                            wqk[:, kt, g * 128:(g + 1) * 128],
                            xT[:, kt, csl],
                            start=(kt == 0), stop=(kt == KT - 1))
                for i in range(2):
                    qk_post(g, 2 * cp + i, ps2[:, i, :])

            def qk_chunk(g, ch):
                """single-chunk qkv on the misc tag (mid-stream filler)."""
                _g_tiles(g)
                csl = slice(ch * 512, (ch + 1) * 512)
                ps_qk = psp.tile([128, 512], F32, tag="misc", bufs=2,
                                 name="ps_qk")
                for kt in range(KT):
                    nc.tensor.matmul(
                        ps_qk,
                        wqk[:, kt, g * 128:(g + 1) * 128],
                        xT[:, kt, csl],
                        start=(kt == 0), stop=(kt == KT - 1))
                qk_post(g, ch, ps_qk)

            def ln_group(g, eng):
                """rstd for group g: transposed ssq matmuls land ms token-on-
                partitions (no reshape DMA); Newton rsqrt on DVE; one scatter
                + two replicating loads on `eng`'s DMA queue."""
                ps_ms = psp.tile([128, 512], F32, tag="misc", bufs=2,
                                 name="ps_mst")
                msv = ps_ms.rearrange("p (c j) -> p c j", j=2)
                for tt in range(NT):
                    nc.tensor.matmul(
                        msv[:, tt, :],
                        sq_g[g][:, tt // 4, (tt % 4) * 128:(tt % 4 + 1) * 128],
                        O2, start=True, stop=True)
                nt_in = wp.tile([128, 32], F32, tag="nt_in", bufs=2)
                nc.vector.tensor_copy(nt_in, ps_ms[:, 0:32])
                t = wp.tile([128, 32], F32, tag="nt_t", bufs=2)
                nc.vector.tensor_scalar(t, nt_in, 1.0 / D, EPS, ALU.mult,
                                        ALU.add)
                y = wp.tile([128, 32], F32, tag="nt_y", bufs=2)
                a = wp.tile([128, 32], F32, tag="nt_a", bufs=2)
                nc.vector.tensor_scalar(y, t, float(_RC[0]), float(_RC[1]),
                                        ALU.mult, ALU.add)
                nc.vector.tensor_mul(y, y, t)
                nc.vector.tensor_scalar_add(y, y, float(_RC[2]))
                nc.vector.tensor_mul(y, y, t)
                nc.vector.tensor_scalar_add(y, y, float(_RC[3]))
                nc.vector.tensor_scalar(y, y, 0.1, 2.4, ALU.max, ALU.min)
                nt_out = wp.tile([128, 2, 16], BF16, tag="nt_out", bufs=2)
                for it in range(2):
                    nc.vector.tensor_mul(a, y, y)
                    nc.vector.tensor_mul(a, a, t)
                    nc.vector.tensor_scalar(a, a, -0.5, 1.5, ALU.mult, ALU.add)
                    if it < 1:
                        nc.vector.tensor_mul(y, y, a)
                    else:
                        nc.vector.tensor_mul(
                            nt_out.rearrange("p j c -> p c j"),
                            y.rearrange("p (c j) -> p c j", j=2),
                            a.rearrange("p (c j) -> p c j", j=2))
                # transpose on PE so both DMA hops stay contiguous:
                # rows_sb[(j,c), tok] -> DRAM rows -> 64-way replicated load
                ps_tr = psp.tile([128, 512], F32, tag="misc", bufs=2,
                                 name="ps_tr")
                nc.tensor.transpose(ps_tr.bitcast(BF16)[0:32, 0:128],
                                    nt_out.rearrange("p j c -> p (j c)"),
                                    eye128)
                rows_sb = wp.tile([32, 128], BF16, tag="rows", bufs=2)
                nc.vector.tensor_copy(rows_sb,
                                      ps_tr.bitcast(BF16)[0:32, 0:128])
                eng.dma_start(out=rstd_g[g].rearrange("j c p -> (j c) p"),
                              in_=rows_sb)
                for j in range(2):
                    row = rstd_g[g][j:j + 1, :, :].rearrange("j c p -> j (c p)")
                    bc = bass.AP(tensor=row.tensor, offset=row.offset,
                                 ap=[[0, 64]] + list(row.ap[1:]))
                    eng.dma_start(out=rb_g[g][64 * j:64 * (j + 1), :], in_=bc)

            def apply_chunk(g, ch):
                """multiply qkt chunk by its per-token rstd broadcast."""
                csl = slice(ch * 512, (ch + 1) * 512)
                nc.vector.tensor_mul(qkt[0:64, g, csl], qkt[0:64, g, csl],
                                     rb_g[g][0:64, csl])
                nc.vector.tensor_mul(qkt[64:128, g, csl], qkt[64:128, g, csl],
                                     rb_g[g][64:128, csl])
                is_q = (g % 2 == 0)
                gcol = None
                if is_q and has_qgamma:
                    gcol = gamma_c[:, 0:1]
                elif not is_q and has_kgamma:
                    gcol = gamma_c[:, 1:2]
                bcol = None
                if is_q and has_qbeta:
                    bcol = beta_c[:, 0:1]
                elif not is_q and has_kbeta:
                    bcol = beta_c[:, 1:2]
                if gcol is not None:
                    nc.vector.tensor_scalar_mul(qkt[:, g, csl],
                                                qkt[:, g, csl], gcol)
                if bcol is not None:
                    nc.vector.tensor_scalar_add(qkt[:, g, csl],
                                                qkt[:, g, csl], bcol)

            def v_feats(tt):
                """v token-major projection for token tile tt."""
                tsl = slice(tt * 128, (tt + 1) * 128)
                ps_v = psp.tile([128, 512], F32, tag="misc", bufs=2,
                                name="ps_v")
                for kt in range(KT):
                    nc.tensor.matmul(
                        ps_v[:, 0:HF], xT[:, kt, tsl], wv[:, kt, :],
                        start=(kt == 0),
                        stop=(not has_vbias and kt == KT - 1))
                if has_vbias:
                    nc.tensor.matmul(ps_v[:, 0:HF], ones512[:, 0:128],
                                     bvT, start=False, stop=True)
                nc.vector.tensor_copy(
                    v_sb[:, tt, :, 0:64],
                    ps_v[:, 0:HF].rearrange("p (h d) -> p h d", h=HPC))

            def normalize(pgq, pqc, pouts):
                """divide PV psum by the fused denominator row, write outT.
                (reciprocal_approx_fast misreads PSUM sources - stage the
                denominator row through SBUF first)"""
                qsl = slice(pqc * 512, (pqc + 1) * 512)
                for hp in range(2):
                    p0 = hp * 64
                    ps_o = pouts[hp]
                    den = wp.tile([1, 512], F32, tag="den", bufs=3)
                    nc.vector.tensor_copy(den, ps_o[64:65, :])
                    rec = wp.tile([1, 512], F32, tag="rec", bufs=3)
                    nc.vector.reciprocal_approx_fast(rec, den)
                    rb2 = wp.tile([64, 512], F32, tag="rb2", bufs=3)
                    nc.gpsimd.partition_broadcast(rb2, rec)
                    nc.vector.tensor_mul(outT_n[p0:p0 + 64, pgq, qsl],
                                         ps_o[0:64, :], rb2)

            def proj_tile(tt, tag="misc"):
                """output projection for one token tile. In-stream tiles ride
                the misc psum tag (so the score rotation never waits on the
                proj cast); tail tiles use the freed score tag."""
                tsl = slice(tt * 128, (tt + 1) * 128)
                ostg = wp.tile([128, DIM], BF16, tag="ostg", bufs=3)
                for fn in range(2):
                    fsl = slice(fn * 512, (fn + 1) * 512)
                    ps_p = psp.tile([128, 512], F32, tag=tag,
                                    bufs=2, name="ps_p")
                    for t in range(2):
                        nc.tensor.matmul(ps_p,
                                         outT_n[:, t, tsl],
                                         wpj[:, t, fsl],
                                         start=(t == 0), stop=(t == 1))
                    nc.vector.tensor_copy(ostg[:, fsl], ps_p)
                nc.sync.dma_start(out=out_d.ap()[tsl, :], in_=ostg)

            def unit(gq, qc, prev, fillers=None, shift=6, trail=False):
                """scores+exp for unit (gq, qc). PV of `prev` rides `shift`
                slots behind; `fillers[kt]` emits extra PE work at slot kt;
                with trail=True this unit's own PV rides 1 slot behind on
                misc-tag psum (last unit only)."""
                qg, kg = (0, 1) if gq == 0 else (2, 3)
                qsl = slice(qc * 512, (qc + 1) * 512)
                exp_pair = wp.tile([128, NT, 2, 512], BF16, tag="exp",
                                   bufs=2, name="exp_pair")
                pouts = touts = None
                if prev is not None:
                    pgq, pqc, pexp = prev
                    pouts = [psp.tile([65, 512], F32, tag="pvc", bufs=2,
                                      name=f"ps_o{hp}") for hp in range(2)]
                if trail:
                    touts = [psp.tile([65, 512], F32, tag="pvc", bufs=2,
                                      name=f"ps_t{hp}") for hp in range(2)]

                def pv_kt(kt):
                    for hp in range(2):
                        nc.tensor.matmul(
                            pouts[hp], v_sb[:, kt, 2 * pgq + hp, 0:65],
                            pexp[:, kt, hp, :],
                            start=(kt == 0), stop=(kt == NT - 1))

                def pvt_kt(kt):
                    for hp in range(2):
                        nc.tensor.matmul(
                            touts[hp], v_sb[:, kt, 2 * gq + hp, 0:65],
                            exp_pair[:, kt, hp, :],
                            start=(kt == 0), stop=(kt == NT - 1))

                for kt in range(NT):
                    ktsl = slice(kt * 128, (kt + 1) * 128)
                    ps_s = psp.tile([128, 2, 512], F32, tag="score",
                                    bufs=2, name="ps_s")
                    for hp in range(2):
                        p0 = hp * 64
                        nc.tensor.matmul(ps_s[:, hp, :],
                                         qkt[p0:p0 + 64, kg, ktsl],
                                         qkt[p0:p0 + 64, qg, qsl],
                                         start=True, stop=True)
                    nc.scalar.activation(exp_pair[:, kt, :, :], ps_s,
                                         AF.Exp, scale=SCALE)
                    if prev is not None:
                        if trail:
                            # dense handoff: prev PV two pairs per slot at
                            # slots 2..9, freeing pvc mid-unit for our own
                            if 2 <= kt <= 9:
                                pv_kt(2 * (kt - 2))
                                pv_kt(2 * (kt - 2) + 1)
                            if kt == 10:
                                normalize(pgq, pqc, pouts)
                        elif kt >= shift:
                            pv_kt(kt - shift)
                    if trail and 11 <= kt:
                        pvt_kt(2 * (kt - 11))
                        pvt_kt(2 * (kt - 11) + 1)
                    if fillers is not None and kt in fillers:
                        fillers[kt]()
                if prev is not None and not trail:
                    for kt in range(NT - shift, NT):
                        pv_kt(kt)
                    normalize(pgq, pqc, pouts)
                if trail:
                    for kt in range(10, NT):
                        pvt_kt(kt)
                    normalize(gq, qc, touts)
                return (gq, qc, exp_pair)

            # ---- emission (priority order = emission order) ----
            with nc.named_scope("prologue"):
                for g in (1, 0):                      # k01 then q01
                    for cp in range(NCH // 2):
                        qk_chunk_pair(g, cp)
                    ln_group(g, nc.scalar)            # DMA on idle ACT queue
                for ch in range(NCH):
                    apply_chunk(1, ch)
                    apply_chunk(0, ch)

            with nc.named_scope("attn"):
                u = unit(0, 0, None,
                         fillers={kt: (lambda kt=kt: v_feats(kt))
                                  for kt in range(NT)})
                f01 = {1: lambda: qk_chunk(2, 0), 4: lambda: qk_chunk(2, 1),
                       7: lambda: qk_chunk(2, 2), 10: lambda: qk_chunk(2, 3)}
                u = unit(0, 1, u, fillers=f01)
                f02 = {0: lambda: ln_group(2, nc.gpsimd),
                       2: lambda: qk_chunk(3, 0), 6: lambda: qk_chunk(3, 1),
                       9: lambda: qk_chunk(3, 2), 12: lambda: qk_chunk(3, 3),
                       14: lambda: [apply_chunk(2, c) for c in range(NCH)]}
                u = unit(0, 2, u, fillers=f02)
                f03 = {1: lambda: ln_group(3, nc.gpsimd),
                       9: lambda: [apply_chunk(3, c) for c in range(NCH)]}
                u = unit(0, 3, u, fillers=f03)
                u = unit(1, 0, u)
                u = unit(1, 1, u)
                fp0 = {2: lambda: proj_tile(0), 6: lambda: proj_tile(1),
                       10: lambda: proj_tile(2), 14: lambda: proj_tile(3)}
                u = unit(1, 2, u, fillers=fp0)
                fp1 = {0: lambda: proj_tile(4), 1: lambda: proj_tile(5),
                       10: lambda: proj_tile(6), 14: lambda: proj_tile(7)}
                u = unit(1, 3, u, fillers=fp1, trail=True)
            with nc.named_scope("tail"):
                for tt in range(8, 16):
                    proj_tile(tt, tag="score")

    nc.compile()
    _BUILD_CACHE[key] = nc
    return nc


def _bf16(a):
    return np.ascontiguousarray(a).astype(ml_dtypes.bfloat16)


def kernel(**inputs):
    global LAST_EXEC_NS
    x = np.asarray(inputs["x"], np.float32)
    w_qkv = np.asarray(inputs["w_qkv"], np.float32)
    b_qkv = np.asarray(inputs["b_qkv"], np.float32)
    q_gamma = np.asarray(inputs["q_gamma"], np.float32)
    q_beta = np.asarray(inputs["q_beta"], np.float32)
    k_gamma = np.asarray(inputs["k_gamma"], np.float32)
    k_beta = np.asarray(inputs["k_beta"], np.float32)
    w_proj = np.asarray(inputs["w_proj"], np.float32)
    b_proj = np.asarray(inputs["b_proj"], np.float32)

    has_qkbias = bool(np.any(b_qkv[0:2 * DIM] != 0.0))
    has_qgamma = not bool(np.all(q_gamma == 1.0))
    has_kgamma = not bool(np.all(k_gamma == 1.0))
    has_qbeta = bool(np.any(q_beta != 0.0))
    has_kbeta = bool(np.any(k_beta != 0.0))
    has_vbias = bool(np.any(b_qkv[2 * DIM:3 * DIM] != 0.0))
    nc = _build(has_qkbias, has_qgamma, has_kgamma, has_qbeta, has_kbeta,
                has_vbias)

    # shared constants
    Cd = np.eye(D, dtype=np.float32) - 1.0 / D   # centering matrix (folded
    eye128 = np.eye(128, dtype=np.float32)       # into the qkv weights below)
    gamma_cols = np.stack([np.tile(q_gamma, 2), np.tile(k_gamma, 2)],
                          axis=1).astype(np.float32)
    ones512 = np.ones((1, 512), np.float32)
    beta_cols = np.stack([np.tile(q_beta, 2), np.tile(k_beta, 2)],
                         axis=1).astype(np.float32)
    # pre-center the q/k projection weights and biases per head:
    # LN(Wx+b) centering is linear, so fold (I - J/64) into W and b
    w_qkv = w_qkv.copy()
    b_qkv = b_qkv.copy()
    for h in range(2 * H):            # 16 q heads then 16 k heads
        rs = slice(h * D, (h + 1) * D)
        w_qkv[rs] = Cd @ w_qkv[rs]
        b_qkv[rs] = Cd @ b_qkv[rs]

    in_maps = []
    for c in range(N_CORES):
        b, hg = divmod(c, 4)
        rows = slice(hg * HF, (hg + 1) * HF)
        q_l = w_qkv[0 * DIM:1 * DIM][rows]           # [256, 1024]
        k_l = w_qkv[1 * DIM:2 * DIM][rows]
        v_l = w_qkv[2 * DIM:3 * DIM][rows]
        bq_l = b_qkv[0 * DIM:1 * DIM][rows]
        bk_l = b_qkv[1 * DIM:2 * DIM][rows]
        bv_l = b_qkv[2 * DIM:3 * DIM][rows]
        # group order [q01 | k01 | q23 | k23]
        wqk_rows = np.concatenate([q_l[:128], k_l[:128],
                                   q_l[128:], k_l[128:]], 0)
        bqk_cols = np.stack([bq_l[:128], bk_l[:128], bq_l[128:], bk_l[128:]],
                            axis=1).astype(np.float32)
        def _pm(a, k):
            # [k*128, c] -> partition-major [128, k, c]
            return _bf16(np.ascontiguousarray(
                a.reshape(k, 128, a.shape[1]).transpose(1, 0, 2)))
        m = {
            "xT": _pm(x[b].T, KT),                        # [128, 8, 2048]
            "wqkT": _pm(wqk_rows.T, KT),                  # [128, 8, 512]
            "wvT": _pm(v_l.T, KT),                        # [128, 8, 256]
            "wpT": _pm(w_proj[:, rows].T, 2),             # [128, 2, 1024]
            "eye128": _bf16(eye128),
        }
        if has_qkbias:
            m["bqk_cols"] = bqk_cols
        if has_vbias:
            m["bvT"] = _bf16(bv_l[None, :])
            m["ones512"] = _bf16(ones512)
        if has_qgamma or has_kgamma:
            m["gamma_cols"] = gamma_cols
        if has_qbeta or has_kbeta:
            m["beta_cols"] = beta_cols
        in_maps.append(m)

    res = run_bass_kernel_spmd(nc, in_maps, core_ids=list(range(N_CORES)),
                               trace=TRACE)
    LAST_EXEC_NS = res.exec_time_ns
    globals()["LAST_RESULTS"] = res

    out = np.zeros((B, N, DIM), np.float32)
    for c in range(N_CORES):
        out[c // 4] += np.asarray(res.results[c]["out_partial"], np.float32)
    out += b_proj[None, None, :]
    return out
